# revision 34
# baseline (speedup 1.0000x reference)
"""Trainium2 Bass kernel for LinearAttention (B=8, S=4096, D=512, H=8, DH=64).

Sharding: data-parallel over batch — core b processes batch element b end-to-end.

Per-core pipeline (matmul inputs in bf16; PSUM accumulation in f32):
  pass A (per 512-wide s-chunk):
    x chunk -> one 1MB DMA -> ACT cast to bf16 -> PE-transpose (bf16, 1 cyc/row)
    qT = Wq^T x^T (psum), phi -> QfT [inner, s]  (bias via ACT per-partition bias)
    k  = x Wk + bk (rank-1 ones-row matmul), phi -> Kf [s, inner]
    v  = x Wv; +bv fused in the psum->sbuf copy -> v' [128, pair, 128+ones]
    KV accumulated per head pair in [128, 129] psum tiles (col 128 = Ksum)
  pass B (per 512-wide s-slice):
    denT = Ksum_bd^T Qf^T in [8, s] layout (block-diag lhsT) + eps rank-1 matmul
    Z^T = 1/denT via DVE recip -> bf16
    Zrep via selector matmul (E8), kept in PSUM
    O^T = KV^T @ QfT per head pair (block-diagonal kvsb)
    OT = O^T * Zrep (DVE, both operands read from PSUM) -> bf16
    out = OT^T Wo + bo (rank-1 matmul into same psum) -> DMA PSUM -> HBM direct
"""

import os
import sys

import numpy as np

for _p in ("/opt/trn_rl_repo",):
    if os.path.isdir(_p) and _p not in sys.path:
        sys.path.insert(0, _p)

from contextlib import ExitStack

import concourse.bass as bass
import concourse.mybir as mybir
import concourse.tile as tile
from concourse.bass_utils import run_bass_kernel_spmd
from concourse.masks import make_identity
from concourse import library_config

B, S, D = 8, 4096, 512
H, DH = 8, 64
INNER = H * DH  # 512
EPS = 1e-6

F32 = mybir.dt.float32
F32R = mybir.dt.float32r
BF16 = mybir.dt.bfloat16
AF = mybir.ActivationFunctionType
ALU = mybir.AluOpType

# matmul input dtype: "bf16" (full-rate, ~5e-3 max-rel) or "f32r" (full-rate for
# moving dims >= 256, ~3e-4) — tolerance gate is 2e-2.
MM_DTYPE = os.environ.get("LINATTN_MM_DTYPE", "bf16")
DT_MM = {"bf16": BF16, "f32r": F32R, "f32": F32}[MM_DTYPE]


def _linattn_body(ctx: ExitStack, tc: "tile.TileContext", io: dict, s_total: int, reps: int = 1):
    nc = tc.nc
    NT = s_total // 128  # s-tiles
    NCH = s_total // 512  # pass-A chunks
    NSL = s_total // 512  # pass-B slices (512-wide)

    x_d = io["x"]
    out_d = io["out"]

    singles = ctx.enter_context(tc.tile_pool(name="singles", bufs=1))

    # ---- constants first (gpsimd): ident needed by the very first transposes ----
    ident = singles.tile([128, 128], DT_MM, name="ident", tag="ident")
    make_identity(nc, ident[:])

    # ---- weights: [128, 4, 512] in DT_MM; chunk c holds rows c*128..(c+1)*128 ----
    # One 1MB DMA per weight on the scalar queue (sync queue is reserved for x /
    # out traffic so x chunk 0 starts at t=0); DVE copies round into DT_MM.
    # NOTE: staging pools stay open for the whole kernel — closing them lets
    # pass-A pools reuse their SBUF, and the resulting WAR dependencies
    # serialize the first x DMA behind the last weight cast (~25us stall).
    # Wv/Wo are emitted later (inside the pass-A pipeline) so their DVE casts
    # don't sit ahead of the chunk-0 xT copies in the DVE queue.
    w_sb = {}
    wraw_pool = ctx.enter_context(tc.tile_pool(name="wraw_pool", bufs=2))

    w_raw = {}

    def emit_weight_dma(name, queue):
        raw = wraw_pool.tile([128, 4, INNER], F32, tag="wraw")
        queue.dma_start(out=raw[:], in_=io[name].rearrange("(c p) n -> p c n", p=128))
        w_raw[name] = raw

    def emit_weight_cast(name):
        raw = w_raw.pop(name)
        t = singles.tile([128, 4, INNER], DT_MM, name=f"{name}_sb", tag=f"{name}_sb")
        for c in range(4):
            nc.vector.tensor_copy(out=t[:, c, :], in_=raw[:, c, :])
        w_sb[name] = t

    def emit_weight(name, queue=None):
        emit_weight_dma(name, queue or nc.scalar)
        emit_weight_cast(name)

    emit_weight("Wq")
    # Wk DMA starts now on the (idle) gpsimd queue, in parallel with Wq on the
    # scalar queue; its DVE cast is emitted later, after the chunk-0/1 xT
    # copies, so its data-wait doesn't block them (engine queues are in-order)
    emit_weight_dma("Wk", nc.gpsimd)
    # bias rows for rank-1 PE bias adds (both on partition 0)
    brow_raw = wraw_pool.tile([1, 2, INNER], F32, tag="brow_raw")
    nc.gpsimd.dma_start(out=brow_raw[:, 0, :], in_=io["bk"].rearrange("(a n) -> a n", a=1))
    nc.gpsimd.dma_start(out=brow_raw[:, 1, :], in_=io["bo"].rearrange("(a n) -> a n", a=1))
    brow = singles.tile([1, 2, INNER], DT_MM, name="brow", tag="brow")
    nc.vector.tensor_copy(out=brow[:], in_=brow_raw[:])
    bk_row, bo_row = brow[:, 0, :], brow[:, 1, :]

    # ---- biases ----
    bq_sb = singles.tile([128, 4], F32, name="bq_sb", tag="bq_sb")
    nc.gpsimd.dma_start(out=bq_sb[:], in_=io["bq"].rearrange("(c p) -> p c", p=128))
    # bv/bo replicated across partitions via partition-step-0 DMA (DVE add operands)
    rep = {}
    for nm in ("bv", "bo"):
        t = singles.tile([128, INNER], F32, name=f"{nm}_rep", tag=f"{nm}_rep")
        ap = io[nm]
        nc.gpsimd.dma_start(
            out=t[:],
            in_=bass.AP(tensor=ap.tensor, offset=ap.offset, ap=[[0, 128]] + list(ap.ap)),
        )
        rep[nm] = t
    bv_rep, bo_rep = rep["bv"], rep["bo"]

    # ---- more constants ----
    # GPSIMD memset/affine_select cannot write f32r: stage in F32, DVE-copy over.
    ones_vcol = singles.tile([128, 4, 1], F32, name="ones_vcol", tag="ones_vcol")
    ones_col = singles.tile([1, 128], DT_MM, name="ones_col", tag="ones_col")
    ones512 = singles.tile([1, 512], DT_MM, name="ones512", tag="ones512")
    eps_col = singles.tile([1, 8], DT_MM, name="eps_col", tag="eps_col")
    e8 = singles.tile([8, 4, 128], DT_MM, name="e8", tag="e8")
    if True:
        cst = ctx.enter_context(tc.tile_pool(name="const_stage", bufs=4))
        nc.gpsimd.memset(ones_vcol[:], 1.0)
        st1 = cst.tile([1, 512], F32, tag="st1")
        nc.gpsimd.memset(st1[:], 1.0)
        nc.vector.tensor_copy(out=ones512[:], in_=st1[:])
        nc.vector.tensor_copy(out=ones_col[:], in_=st1[:, 0:128])
        ste = cst.tile([1, 8], F32, tag="ste")
        nc.gpsimd.memset(ste[:], EPS)
        nc.vector.tensor_copy(out=eps_col[:], in_=ste[:])
        st8 = cst.tile([8, 4, 128], F32, tag="st8")
        nc.gpsimd.memset(st8[:], 0.0)
        nc.gpsimd.affine_select(
            out=st8[:, :, 0:64], in_=st8[:, :, 0:64], compare_op=ALU.not_equal, fill=1.0,
            base=0, pattern=[[-2, 4], [0, 64]], channel_multiplier=1,
        )
        nc.gpsimd.affine_select(
            out=st8[:, :, 64:128], in_=st8[:, :, 64:128], compare_op=ALU.not_equal, fill=1.0,
            base=-1, pattern=[[-2, 4], [0, 64]], channel_multiplier=1,
        )
        nc.vector.tensor_copy(out=e8[:], in_=st8[:])

    # ---- persistent per-core buffers ----
    qfT = singles.tile([128, 4, s_total], DT_MM, name="qfT", tag="qfT")  # [inner, s]
    kvsb = singles.tile([128, 4, 128], DT_MM, name="kvsb", tag="kvsb")  # block-diag per pair
    # v' staging buffers (manual 3-deep rotation so the ones column is written once)
    vqs = [singles.tile([128, 4, 129], DT_MM, name=f"vq{i}", tag=f"vq{i}") for i in range(3)]
    for i in range(3):
        nc.vector.tensor_copy(out=vqs[i][:, :, 128:129], in_=ones_vcol[:])
    # block-diag Ksum rhs: [128, pair, 8]; pair c: rows 0-63 -> col 2c, rows 64-127 -> col 2c+1
    ksum_bd = singles.tile([128, 4, 8], DT_MM, name="ksum_bd", tag="ksum_bd")
    if True:
        kbz = ctx.enter_context(tc.tile_pool(name="kbz_stage", bufs=2))
        stz = kbz.tile([128, 4, 8], F32, tag="stz")
        nc.gpsimd.memset(stz[:], 0.0)
        nc.vector.tensor_copy(out=ksum_bd[:], in_=stz[:])
        stz2 = kbz.tile([128, 4, 128], F32, tag="stz2")
        nc.gpsimd.memset(stz2[:], 0.0)
        nc.vector.tensor_copy(out=kvsb[:], in_=stz2[:])

    def _passes():
        # =================== PASS A ===================
        with ExitStack() as actx:
            x_pool = actx.enter_context(tc.tile_pool(name="x_pool", bufs=2))
            xbf_pool = actx.enter_context(tc.tile_pool(name="xbf_pool", bufs=3))
            xT_pool = actx.enter_context(tc.tile_pool(name="xT_pool", bufs=2))
            er_pool = actx.enter_context(tc.tile_pool(name="er_pool", bufs=6))
            kf_pool = actx.enter_context(tc.tile_pool(name="kf_pool", bufs=3))
            v_pool = actx.enter_context(tc.tile_pool(name="v_pool", bufs=3))
            ps_a = actx.enter_context(tc.tile_pool(name="ps_a", bufs=4, space="PSUM"))
            ps_acc = actx.enter_context(tc.tile_pool(name="ps_acc", bufs=1, space="PSUM"))

            # KV accumulators per head pair (one PSUM bank each — interleaved
            # accumulation groups must not share a bank):
            # kvq[j] = cols 0-127 pair j's v cols, col 128 = Ksum
            kvq = [
                ps_acc.tile([128, 129], F32, name=f"kvq_{j}", tag=f"kvq_{j}")[:]
                for j in range(4)
            ]

            xT_live = {}

            def transpose_stage(ich):
                xc = x_pool.tile([128, 4, D], F32, tag="x")
                if ich == 0:
                    # chunk 0 arrives tile-by-tile so the first transpose can
                    # start ~4us earlier than a monolithic 1MB transfer allows
                    for it in range(4):
                        nc.sync.dma_start(
                            out=xc[:, it, :], in_=x_d[it * 128 : (it + 1) * 128, :]
                        )
                else:
                    nc.sync.dma_start(
                        out=xc[:],
                        in_=x_d[ich * 512 : (ich + 1) * 512, :].rearrange("(t p) d -> p t d", p=128),
                    )
                xT_t = xT_pool.tile([128, 4, 512], DT_MM, tag="xT")
                # ---- cast to bf16 on ACT, transpose chunk on PE (1 cyc/row) ----
                for it in range(4):
                    xbf = xbf_pool.tile([128, D], DT_MM, tag="xbf")
                    nc.scalar.activation(xbf[:], xc[:, it, :], AF.Copy)
                    xps = ps_a.tile([128, 4, 128], DT_MM, tag="ps")
                    for c in range(4):
                        nc.tensor.transpose(xps[:, c, :], xbf[:, c * 128 : (c + 1) * 128], ident[:])
                    nc.vector.tensor_copy(out=xT_t[:, :, it * 128 : (it + 1) * 128], in_=xps[:])
                xT_live[ich] = xT_t

            def qkv_stage(ich):
                xT_t = xT_live.pop(ich)
                # ---- qT = Wq^T xT, phi -> QfT ----
                for ci in range(4):
                    qps = ps_a.tile([128, 512], F32, tag="ps")
                    for cd in range(4):
                        nc.tensor.matmul(
                            qps[:],
                            lhsT=w_sb["Wq"][:, cd, ci * 128 : (ci + 1) * 128],
                            rhs=xT_t[:, cd, :],
                            start=(cd == 0),
                            stop=(cd == 3),
                        )
                    e_t = er_pool.tile([128, 512], F32, tag="er")
                    r_t = er_pool.tile([128, 512], F32, tag="er")
                    nc.scalar.activation(e_t[:], qps[:], AF.Exp, bias=bq_sb[:, ci : ci + 1], scale=1.0)
                    nc.scalar.activation(r_t[:], qps[:], AF.Relu, bias=bq_sb[:, ci : ci + 1], scale=1.0)
                    # phi = min(exp(x),1) + relu(x)
                    nc.vector.scalar_tensor_tensor(
                        out=qfT[:, ci, ich * 512 : (ich + 1) * 512],
                        in0=e_t[:],
                        scalar=1.0,
                        in1=r_t[:],
                        op0=ALU.min,
                        op1=ALU.add,
                    )
                # ---- k, v, KV accumulation per s-tile ----
                for it in range(4):
                    ist = ich * 4 + it
                    first, last = (ist == 0), (ist == NT - 1)
                    # k (natural layout) + bias via ones-row matmul
                    kps = ps_a.tile([128, 512], F32, tag="ps")
                    for cd in range(4):
                        nc.tensor.matmul(
                            kps[:],
                            lhsT=xT_t[:, cd, it * 128 : (it + 1) * 128],
                            rhs=w_sb["Wk"][:, cd, :],
                            start=(cd == 0),
                            stop=False,
                        )
                    nc.tensor.matmul(
                        kps[:], lhsT=ones_col[:], rhs=bk_row, start=False, stop=True
                    )
                    e_t = er_pool.tile([128, 512], F32, tag="er")
                    r_t = er_pool.tile([128, 512], F32, tag="er")
                    nc.scalar.activation(e_t[:], kps[:], AF.Exp)
                    nc.scalar.activation(r_t[:], kps[:], AF.Relu)
                    kf = kf_pool.tile([128, 512], DT_MM, tag="kf")
                    nc.vector.scalar_tensor_tensor(
                        out=kf[:], in0=e_t[:], scalar=1.0, in1=r_t[:], op0=ALU.min, op1=ALU.add
                    )
                    # v (natural) with bias fused into the psum->sbuf copy
                    vps = ps_a.tile([128, 512], F32, tag="ps")
                    for cd in range(4):
                        nc.tensor.matmul(
                            vps[:],
                            lhsT=xT_t[:, cd, it * 128 : (it + 1) * 128],
                            rhs=w_sb["Wv"][:, cd, :],
                            start=(cd == 0),
                            stop=(cd == 3),
                        )
                    vq = vqs[ist % 3]
                    nc.vector.tensor_add(
                        out=vq[:, :, 0:128],
                        in0=vps[:].rearrange("p (g n) -> p g n", g=4),
                        in1=bv_rep[:].rearrange("p (g n) -> p g n", g=4),
                    )
                    # KV accumulation per head pair
                    for j in range(4):
                        nc.tensor.matmul(
                            kvq[j],
                            lhsT=kf[:, j * 128 : (j + 1) * 128],
                            rhs=vq[:, j, :],
                            start=first,
                            stop=last,
                        )

            # software pipeline: transposes run one chunk ahead of Q/K/V so the
            # PE has work while weights load and ACT casts the next chunk
            for ich in range(NCH + 1):
                if ich < NCH:
                    transpose_stage(ich)
                if ich == 1:
                    # late-emitted weights: their DVE casts queue behind the
                    # chunk-0/1 xT copies instead of ahead of them
                    emit_weight_cast("Wk")
                    emit_weight("Wv")
                    emit_weight("Wo")
                if ich >= 1:
                    qkv_stage(ich - 1)

            # ---- extract KV blocks and Ksum (still inside pass-A pool scope) ----
            for h in range(H):
                j, rh = h // 2, (h % 2) * 64
                nc.vector.tensor_copy(
                    out=kvsb[rh : rh + 64, j, rh : rh + 64],
                    in_=kvq[j][rh : rh + 64, rh : rh + 64],
                )
            for c in range(4):
                for half in range(2):
                    nc.vector.tensor_copy(
                        out=ksum_bd[half * 64 : (half + 1) * 64, c, 2 * c + half : 2 * c + half + 1],
                        in_=kvq[c][half * 64 : (half + 1) * 64, 128:129],
                    )

        # ======================= PASS B =======================
        # Phase 1: precompute the replicated 1/den for ALL slices (the serial
        # denT -> Ln -> Exp -> zrep -> copy chain), so phase 2 is a pure
        # OT -> mul -> out stream with no ACT in the critical path.
        with ExitStack() as bctx:
            zsb_pool = bctx.enter_context(tc.tile_pool(name="zsb_pool", bufs=NSL))
            zrep_all = []
            with ExitStack() as pctx:
                dz_ps = pctx.enter_context(tc.tile_pool(name="dz_ps", bufs=2, space="PSUM"))
                zrep_ps = pctx.enter_context(tc.tile_pool(name="zrep_ps", bufs=1, space="PSUM"))
                ztsb_pool = pctx.enter_context(tc.tile_pool(name="ztsb_pool", bufs=2))
                for isl in range(NSL):
                    # denT [8, 512] = sum_c ksum_bd[:,c,:]^T @ qfT[:,c,slice] + eps
                    dz = dz_ps.tile([8, 512], F32, tag="dz")
                    for c in range(4):
                        nc.tensor.matmul(
                            dz[:],
                            lhsT=ksum_bd[:, c, :],
                            rhs=qfT[:, c, isl * 512 : (isl + 1) * 512],
                            start=(c == 0),
                            stop=False,
                        )
                    nc.tensor.matmul(dz[:], lhsT=eps_col[:], rhs=ones512[:], start=False, stop=True)
                    # Z = 1/den as exp(-ln(den)) on ACT (Reciprocal is banned
                    # there; DVE reciprocal is ~6.5ns/elem — too slow at 512 free)
                    lntmp = ztsb_pool.tile([8, 512], F32, tag="lntmp")
                    nc.scalar.activation(lntmp[:], dz[:], AF.Ln)
                    ztsb = ztsb_pool.tile([8, 512], DT_MM, tag="ztsb")
                    nc.scalar.activation(ztsb[:], lntmp[:], AF.Exp, scale=-1.0)
                    # Zrep: replicate Z rows across head d-partitions (E8
                    # selector matmul, 512-wide), staged to SBUF via ACT
                    # (DVE may read only one PSUM operand per op)
                    zrep = zrep_ps.tile([128, 4, 512], F32, tag="zrep")
                    for c in range(4):
                        nc.tensor.matmul(
                            zrep[:, c, :], lhsT=e8[:, c, :], rhs=ztsb[:], start=True, stop=True
                        )
                    zrep_sb = zsb_pool.tile([128, 4, 512], DT_MM, tag="zrep_sb")
                    nc.scalar.activation(zrep_sb[:], zrep[:], AF.Copy)
                    zrep_all.append(zrep_sb)

            # Phase 2: OT -> Z-scale -> out projection, stream-bound
            ot_ps = bctx.enter_context(tc.tile_pool(name="ot_ps", bufs=2, space="PSUM"))
            ow_ps = bctx.enter_context(tc.tile_pool(name="ow_ps", bufs=3, space="PSUM"))
            otsb_pool = bctx.enter_context(tc.tile_pool(name="otsb_pool", bufs=2))
            out_pool = bctx.enter_context(tc.tile_pool(name="out_pool", bufs=3))
            for isl in range(NSL):
                for half in range(2):
                    # O^T per head pair (block-diagonal KV)
                    otps = ot_ps.tile([128, 4, 256], F32, tag="ot")
                    for c in range(4):
                        nc.tensor.matmul(
                            otps[:, c, :],
                            lhsT=kvsb[:, c, :],
                            rhs=qfT[:, c, isl * 512 + half * 256 : isl * 512 + (half + 1) * 256],
                            start=True,
                            stop=True,
                        )
                    otsb = otsb_pool.tile([128, 4, 256], DT_MM, tag="otsb")
                    nc.vector.tensor_mul(
                        out=otsb[:],
                        in0=otps[:],
                        in1=zrep_all[isl][:, :, half * 256 : (half + 1) * 256],
                    )
                    # out = OT^T Wo; +bo fused into the psum->sbuf copy
                    for h2 in range(2):
                        ist = isl * 4 + half * 2 + h2
                        owps = ow_ps.tile([128, 512], F32, tag="ow")
                        for c in range(4):
                            nc.tensor.matmul(
                                owps[:],
                                lhsT=otsb[:, c, h2 * 128 : (h2 + 1) * 128],
                                rhs=w_sb["Wo"][:, c, :],
                                start=(c == 0),
                                stop=(c == 3),
                            )
                        outt = out_pool.tile([128, 512], F32, tag="outt")
                        nc.vector.tensor_add(out=outt[:], in0=owps[:], in1=bo_rep[:])
                        nc.sync.dma_start(out=out_d[ist * 128 : (ist + 1) * 128, :], in_=outt[:])

    if reps == 1:
        _passes()
    else:
        with tc.For_i(0, reps, 1):
            _passes()


def _legalize_waits(nc: "bass.Bass", max_waits: int = 1) -> int:
    """This toolchain's walrus allows at most ONE sync wait per instruction.

    Tile's scheduler attaches several; hoist the extras into standalone
    event-semaphore (pure wait) instructions on the same engine, placed
    immediately before the original — identical blocking semantics since
    waits execute in stream order on the issuing sequencer.
    """
    n_split = 0
    for func in nc.m.functions:
        for block in func.blocks:
            new_insts = []
            for inst in block.instructions:
                si = getattr(inst, "sync_info", None)
                waits = list(si.on_wait) if (si and si.on_wait) else []
                if len(waits) > max_waits:
                    extra, keep = waits[:-max_waits], waits[-max_waits:]
                    for j, w in enumerate(extra):
                        ev = mybir.InstEventSemaphore(
                            name=f"{inst.name}_lw{j}",
                            engine=inst.engine,
                            ins=[],
                            outs=[],
                            sync_info=mybir.SyncInfo(on_wait=[w], on_update=[]),
                        )
                        new_insts.append(ev)
                        n_split += 1
                    si.on_wait = keep
                new_insts.append(inst)
            block.instructions[:] = new_insts
    return n_split




def build_program(s_total: int = S, reps: int = 1) -> "bass.Bass":
    nc = bass.Bass("TRN2", target_bir_lowering=False, debug=False, num_devices=B)
    io = {
        "x": nc.dram_tensor("x", [s_total, D], F32, kind="ExternalInput").ap(),
        "Wq": nc.dram_tensor("Wq", [D, INNER], F32, kind="ExternalInput").ap(),
        "bq": nc.dram_tensor("bq", [INNER], F32, kind="ExternalInput").ap(),
        "Wk": nc.dram_tensor("Wk", [D, INNER], F32, kind="ExternalInput").ap(),
        "bk": nc.dram_tensor("bk", [INNER], F32, kind="ExternalInput").ap(),
        "Wv": nc.dram_tensor("Wv", [D, INNER], F32, kind="ExternalInput").ap(),
        "bv": nc.dram_tensor("bv", [INNER], F32, kind="ExternalInput").ap(),
        "Wo": nc.dram_tensor("Wo", [INNER, D], F32, kind="ExternalInput").ap(),
        "bo": nc.dram_tensor("bo", [D], F32, kind="ExternalInput").ap(),
        "out": nc.dram_tensor("out", [s_total, D], F32, kind="ExternalOutput").ap(),
    }
    with tile.TileContext(nc) as tc:
        with ExitStack() as ctx:
            _linattn_body(ctx, tc, io, s_total, reps=reps)
    return nc


_PROGRAM_CACHE: dict = {}


def _get_program(s_total: int = S) -> "bass.Bass":
    if s_total not in _PROGRAM_CACHE:
        nc = build_program(s_total)
        _legalize_waits(nc)
        _PROGRAM_CACHE[s_total] = nc
    return _PROGRAM_CACHE[s_total]


def _in_maps(inputs: dict) -> list:
    maps = []
    for b in range(B):
        m = {"x": np.ascontiguousarray(inputs["x"][b], dtype=np.float32)}
        for name in ("Wq", "bq", "Wk", "bk", "Wv", "bv", "Wo", "bo"):
            m[name] = np.ascontiguousarray(inputs[name], dtype=np.float32)
        maps.append(m)
    return maps


def run_hw(inputs: dict, trace: bool = False, **kwargs):
    """Run on the 8 NeuronCores. Returns (out [B,S,D], BassKernelResults)."""
    nc = _get_program(S)
    res = run_bass_kernel_spmd(nc, _in_maps(inputs), list(range(B)), trace=trace, **kwargs)
    out = np.stack([res.results[b]["out"] for b in range(B)], axis=0)
    return out, res


def kernel(**inputs) -> np.ndarray:
    out, _ = run_hw(inputs, trace=False)
    return out


def bench_hw(inputs: dict, iters: int = 20, nc_override=None):
    """Time repeated NEFF executions with device-resident inputs.

    Returns (per_iter_ns, out[B,S,D] from the first run). Uses the same
    shard_map lowering as run_bass_via_pjrt, without donation so input
    buffers can be reused across timed calls.
    """
    import time as _time

    import jax
    from jax.sharding import Mesh, NamedSharding, PartitionSpec
    from jax.experimental.shard_map import shard_map

    from concourse import bass2jax
    from concourse.bass2jax import _bass_exec_p, install_neuronx_cc_hook

    install_neuronx_cc_hook()
    nc = nc_override if nc_override is not None else _get_program(S)
    in_maps = _in_maps(inputs)

    partition_name = nc.partition_id_tensor.name if nc.partition_id_tensor else None
    in_names, out_names, out_avals = [], [], []
    for alloc in nc.m.functions[0].allocations:
        if not isinstance(alloc, mybir.MemoryLocationSet):
            continue
        name = alloc.memorylocations[0].name
        if alloc.kind == "ExternalInput":
            if name != partition_name:
                in_names.append(name)
        elif alloc.kind == "ExternalOutput":
            out_names.append(name)
            out_avals.append(
                jax.core.ShapedArray(tuple(alloc.tensor_shape), mybir.dt.np(alloc.dtype))
            )
    n_params = len(in_names)
    all_in_names = in_names + out_names
    if partition_name is not None:
        all_in_names = all_in_names + [partition_name]

    def _body(*args):
        operands = list(args)
        if partition_name is not None:
            operands.append(bass2jax.partition_id_tensor())
        outs = _bass_exec_p.bind(
            *operands,
            out_avals=tuple(out_avals),
            in_names=tuple(all_in_names),
            out_names=tuple(out_names),
            lowering_input_output_aliases=(),
            sim_require_finite=True,
            sim_require_nnan=True,
            nc=nc,
        )
        return tuple(outs)

    devices = jax.devices()[:B]
    mesh = Mesh(np.asarray(devices), ("core",))
    n_outs = len(out_names)
    in_specs = (PartitionSpec("core"),) * (n_params + n_outs)
    out_specs = (PartitionSpec("core"),) * n_outs
    fn = jax.jit(
        shard_map(_body, mesh=mesh, in_specs=in_specs, out_specs=out_specs, check_rep=False)
    )

    sh = NamedSharding(mesh, PartitionSpec("core"))
    concat_in = [
        jax.device_put(
            np.concatenate([np.asarray(in_maps[c][nm])[None] for c in range(B)], axis=0).reshape(
                B * np.asarray(in_maps[0][nm]).shape[0], *np.asarray(in_maps[0][nm]).shape[1:]
            ),
            sh,
        )
        for nm in in_names
    ]
    concat_zeros = [
        jax.device_put(np.zeros((B * a.shape[0], *a.shape[1:]), a.dtype), sh) for a in out_avals
    ]

    out = fn(*concat_in, *concat_zeros)
    jax.block_until_ready(out)
    first = np.asarray(out[0]).reshape(B, *out_avals[0].shape)

    def timed(f, n):
        t0 = _time.perf_counter()
        for _ in range(n):
            r = f(*concat_in, *concat_zeros)
        jax.block_until_ready(r)
        return (_time.perf_counter() - t0) / n

    timed(fn, 3)
    t = min(timed(fn, max(5, iters // 2)) for _ in range(4))
    return int(t * 1e9), first


def build_copy_program(s_total: int = S) -> "bass.Bass":
    """Same I/O signature as the real program, near-zero work: out = x."""
    nc = bass.Bass("TRN2", target_bir_lowering=False, debug=False, num_devices=B)
    io = {}
    io["x"] = nc.dram_tensor("x", [s_total, D], F32, kind="ExternalInput").ap()
    for nm, shp in (("Wq", [D, INNER]), ("bq", [INNER]), ("Wk", [D, INNER]), ("bk", [INNER]),
                    ("Wv", [D, INNER]), ("bv", [INNER]), ("Wo", [INNER, D]), ("bo", [D])):
        io[nm] = nc.dram_tensor(nm, shp, F32, kind="ExternalInput").ap()
    out_d = nc.dram_tensor("out", [s_total, D], F32, kind="ExternalOutput").ap()
    from contextlib import ExitStack as _ES
    with tile.TileContext(nc) as tc:
        with _ES() as ctx:
            pool = ctx.enter_context(tc.tile_pool(name="cp", bufs=4))
            for i in range(s_total // 128):
                t = pool.tile([128, D], F32, tag="cp")
                sl = slice(i * 128, (i + 1) * 128)
                nc.sync.dma_start(out=t[:], in_=io["x"][sl])
                nc.sync.dma_start(out=out_d[sl], in_=t[:])
    _legalize_waits(nc)
    return nc


# revision 35
# speedup vs baseline: 1.0594x; 1.0594x over previous
"""Trainium2 Bass kernel for LinearAttention (B=8, S=4096, D=512, H=8, DH=64).

Sharding: data-parallel over batch — core b processes batch element b end-to-end.

Per-core pipeline (matmul inputs in bf16; PSUM accumulation in f32):
  pass A (per 512-wide s-chunk):
    x chunk -> one 1MB DMA -> ACT cast to bf16 -> PE-transpose (bf16, 1 cyc/row)
    qT = Wq^T x^T (psum), phi -> QfT [inner, s]  (bias via ACT per-partition bias)
    k  = x Wk + bk (rank-1 ones-row matmul), phi -> Kf [s, inner]
    v  = x Wv; +bv fused in the psum->sbuf copy -> v' [128, pair, 128+ones]
    KV accumulated per head pair in [128, 129] psum tiles (col 128 = Ksum)
  pass B (per 512-wide s-slice):
    denT = Ksum_bd^T Qf^T in [8, s] layout (block-diag lhsT) + eps rank-1 matmul
    Z^T = 1/denT via DVE recip -> bf16
    Zrep via selector matmul (E8), kept in PSUM
    O^T = KV^T @ QfT per head pair (block-diagonal kvsb)
    OT = O^T * Zrep (DVE, both operands read from PSUM) -> bf16
    out = OT^T Wo + bo (rank-1 matmul into same psum) -> DMA PSUM -> HBM direct
"""

import os
import sys

import numpy as np

for _p in ("/opt/trn_rl_repo",):
    if os.path.isdir(_p) and _p not in sys.path:
        sys.path.insert(0, _p)

from contextlib import ExitStack

import concourse.bass as bass
import concourse.mybir as mybir
import concourse.tile as tile
from concourse.bass_utils import run_bass_kernel_spmd
from concourse.masks import make_identity
from concourse import library_config

B, S, D = 8, 4096, 512
H, DH = 8, 64
INNER = H * DH  # 512
EPS = 1e-6

F32 = mybir.dt.float32
F32R = mybir.dt.float32r
BF16 = mybir.dt.bfloat16
AF = mybir.ActivationFunctionType
ALU = mybir.AluOpType

# matmul input dtype: "bf16" (full-rate, ~5e-3 max-rel) or "f32r" (full-rate for
# moving dims >= 256, ~3e-4) — tolerance gate is 2e-2.
MM_DTYPE = os.environ.get("LINATTN_MM_DTYPE", "bf16")
DT_MM = {"bf16": BF16, "f32r": F32R, "f32": F32}[MM_DTYPE]


def _linattn_body(ctx: ExitStack, tc: "tile.TileContext", io: dict, s_total: int, reps: int = 1):
    nc = tc.nc
    NT = s_total // 128  # s-tiles
    NCH = s_total // 512  # pass-A chunks
    NSL = s_total // 512  # pass-B slices (512-wide)

    x_d = io["x"]
    out_d = io["out"]

    singles = ctx.enter_context(tc.tile_pool(name="singles", bufs=1))

    # ---- constants first (gpsimd): ident needed by the very first transposes ----
    ident = singles.tile([128, 128], DT_MM, name="ident", tag="ident")
    make_identity(nc, ident[:])

    # ---- weights: [128, 4, 512] in DT_MM; chunk c holds rows c*128..(c+1)*128 ----
    # One 1MB DMA per weight on the scalar queue (sync queue is reserved for x /
    # out traffic so x chunk 0 starts at t=0); DVE copies round into DT_MM.
    # NOTE: staging pools stay open for the whole kernel — closing them lets
    # pass-A pools reuse their SBUF, and the resulting WAR dependencies
    # serialize the first x DMA behind the last weight cast (~25us stall).
    # Wv/Wo are emitted later (inside the pass-A pipeline) so their DVE casts
    # don't sit ahead of the chunk-0 xT copies in the DVE queue.
    w_sb = {}
    wraw_pool = ctx.enter_context(tc.tile_pool(name="wraw_pool", bufs=2))

    w_raw = {}

    def emit_weight_dma(name, queue):
        raw = wraw_pool.tile([128, 4, INNER], F32, tag="wraw")
        queue.dma_start(out=raw[:], in_=io[name].rearrange("(c p) n -> p c n", p=128))
        w_raw[name] = raw

    def emit_weight_cast(name):
        raw = w_raw.pop(name)
        t = singles.tile([128, 4, INNER], DT_MM, name=f"{name}_sb", tag=f"{name}_sb")
        for c in range(4):
            nc.vector.tensor_copy(out=t[:, c, :], in_=raw[:, c, :])
        w_sb[name] = t

    def emit_weight(name, queue=None):
        emit_weight_dma(name, queue or nc.scalar)
        emit_weight_cast(name)

    emit_weight("Wq")
    # Wk DMA starts now on the (idle) gpsimd queue, in parallel with Wq on the
    # scalar queue; its DVE cast is emitted later, after the chunk-0/1 xT
    # copies, so its data-wait doesn't block them (engine queues are in-order)
    emit_weight_dma("Wk", nc.gpsimd)
    # bias rows for rank-1 PE bias adds (both on partition 0)
    brow_raw = wraw_pool.tile([1, 2, INNER], F32, tag="brow_raw")
    nc.gpsimd.dma_start(out=brow_raw[:, 0, :], in_=io["bk"].rearrange("(a n) -> a n", a=1))
    nc.gpsimd.dma_start(out=brow_raw[:, 1, :], in_=io["bo"].rearrange("(a n) -> a n", a=1))
    brow = singles.tile([1, 2, INNER], DT_MM, name="brow", tag="brow")
    nc.vector.tensor_copy(out=brow[:], in_=brow_raw[:])
    bk_row, bo_row = brow[:, 0, :], brow[:, 1, :]

    # ---- biases ----
    bq_sb = singles.tile([128, 4], F32, name="bq_sb", tag="bq_sb")
    nc.gpsimd.dma_start(out=bq_sb[:], in_=io["bq"].rearrange("(c p) -> p c", p=128))
    # bv/bo replicated across partitions via partition-step-0 DMA (DVE add operands)
    rep = {}
    for nm in ("bv", "bo"):
        t = singles.tile([128, INNER], F32, name=f"{nm}_rep", tag=f"{nm}_rep")
        ap = io[nm]
        nc.gpsimd.dma_start(
            out=t[:],
            in_=bass.AP(tensor=ap.tensor, offset=ap.offset, ap=[[0, 128]] + list(ap.ap)),
        )
        rep[nm] = t
    bv_rep, bo_rep = rep["bv"], rep["bo"]

    # ---- more constants ----
    # GPSIMD memset/affine_select cannot write f32r: stage in F32, DVE-copy over.
    ones_vcol = singles.tile([128, 4, 1], F32, name="ones_vcol", tag="ones_vcol")
    ones_col = singles.tile([1, 128], DT_MM, name="ones_col", tag="ones_col")
    ones512 = singles.tile([1, 512], DT_MM, name="ones512", tag="ones512")
    eps_col = singles.tile([1, 8], DT_MM, name="eps_col", tag="eps_col")
    e8 = singles.tile([8, 4, 128], DT_MM, name="e8", tag="e8")
    if True:
        cst = ctx.enter_context(tc.tile_pool(name="const_stage", bufs=4))
        nc.gpsimd.memset(ones_vcol[:], 1.0)
        st1 = cst.tile([1, 512], F32, tag="st1")
        nc.gpsimd.memset(st1[:], 1.0)
        nc.vector.tensor_copy(out=ones512[:], in_=st1[:])
        nc.vector.tensor_copy(out=ones_col[:], in_=st1[:, 0:128])
        ste = cst.tile([1, 8], F32, tag="ste")
        nc.gpsimd.memset(ste[:], EPS)
        nc.vector.tensor_copy(out=eps_col[:], in_=ste[:])
        st8 = cst.tile([8, 4, 128], F32, tag="st8")
        nc.gpsimd.memset(st8[:], 0.0)
        nc.gpsimd.affine_select(
            out=st8[:, :, 0:64], in_=st8[:, :, 0:64], compare_op=ALU.not_equal, fill=1.0,
            base=0, pattern=[[-2, 4], [0, 64]], channel_multiplier=1,
        )
        nc.gpsimd.affine_select(
            out=st8[:, :, 64:128], in_=st8[:, :, 64:128], compare_op=ALU.not_equal, fill=1.0,
            base=-1, pattern=[[-2, 4], [0, 64]], channel_multiplier=1,
        )
        nc.vector.tensor_copy(out=e8[:], in_=st8[:])

    # ---- persistent per-core buffers ----
    qfT = singles.tile([128, 4, s_total], DT_MM, name="qfT", tag="qfT")  # [inner, s]
    kvsb = singles.tile([128, 4, 128], DT_MM, name="kvsb", tag="kvsb")  # block-diag per pair
    # v' staging buffers (manual 3-deep rotation so the ones column is written once)
    vqs = [singles.tile([128, 4, 129], DT_MM, name=f"vq{i}", tag=f"vq{i}") for i in range(3)]
    for i in range(3):
        nc.vector.tensor_copy(out=vqs[i][:, :, 128:129], in_=ones_vcol[:])
    # block-diag Ksum rhs: [128, pair, 8]; pair c: rows 0-63 -> col 2c, rows 64-127 -> col 2c+1
    ksum_bd = singles.tile([128, 4, 8], DT_MM, name="ksum_bd", tag="ksum_bd")
    if True:
        kbz = ctx.enter_context(tc.tile_pool(name="kbz_stage", bufs=2))
        stz = kbz.tile([128, 4, 8], F32, tag="stz")
        nc.gpsimd.memset(stz[:], 0.0)
        nc.vector.tensor_copy(out=ksum_bd[:], in_=stz[:])
        stz2 = kbz.tile([128, 4, 128], F32, tag="stz2")
        nc.gpsimd.memset(stz2[:], 0.0)
        nc.vector.tensor_copy(out=kvsb[:], in_=stz2[:])

    def _passes():
        # =================== PASS A ===================
        with ExitStack() as actx:
            x_pool = actx.enter_context(tc.tile_pool(name="x_pool", bufs=2))
            xbf_pool = actx.enter_context(tc.tile_pool(name="xbf_pool", bufs=3))
            xT_pool = actx.enter_context(tc.tile_pool(name="xT_pool", bufs=2))
            er_pool = actx.enter_context(tc.tile_pool(name="er_pool", bufs=6))
            kf_pool = actx.enter_context(tc.tile_pool(name="kf_pool", bufs=3))
            v_pool = actx.enter_context(tc.tile_pool(name="v_pool", bufs=3))
            ps_a = actx.enter_context(tc.tile_pool(name="ps_a", bufs=4, space="PSUM"))
            ps_acc = actx.enter_context(tc.tile_pool(name="ps_acc", bufs=1, space="PSUM"))

            # KV accumulators per head pair (one PSUM bank each — interleaved
            # accumulation groups must not share a bank):
            # kvq[j] = cols 0-127 pair j's v cols, col 128 = Ksum
            kvq = [
                ps_acc.tile([128, 129], F32, name=f"kvq_{j}", tag=f"kvq_{j}")[:]
                for j in range(4)
            ]

            xT_live = {}

            def transpose_stage(ich):
                xc = x_pool.tile([128, 4, D], F32, tag="x")
                if ich == 0:
                    # chunk 0 arrives tile-by-tile so the first transpose can
                    # start ~4us earlier than a monolithic 1MB transfer allows
                    for it in range(4):
                        nc.sync.dma_start(
                            out=xc[:, it, :], in_=x_d[it * 128 : (it + 1) * 128, :]
                        )
                else:
                    nc.sync.dma_start(
                        out=xc[:],
                        in_=x_d[ich * 512 : (ich + 1) * 512, :].rearrange("(t p) d -> p t d", p=128),
                    )
                xT_t = xT_pool.tile([128, 4, 512], DT_MM, tag="xT")
                # ---- cast to bf16 on ACT, transpose chunk on PE (1 cyc/row) ----
                for it in range(4):
                    xbf = xbf_pool.tile([128, D], DT_MM, tag="xbf")
                    nc.scalar.activation(xbf[:], xc[:, it, :], AF.Copy)
                    xps = ps_a.tile([128, 4, 128], DT_MM, tag="ps")
                    for c in range(4):
                        nc.tensor.transpose(xps[:, c, :], xbf[:, c * 128 : (c + 1) * 128], ident[:])
                    nc.vector.tensor_copy(out=xT_t[:, :, it * 128 : (it + 1) * 128], in_=xps[:])
                xT_live[ich] = xT_t

            def qkv_stage(ich):
                xT_t = xT_live.pop(ich)
                # ---- qT = Wq^T xT, phi -> QfT ----
                for ci in range(4):
                    qps = ps_a.tile([128, 512], F32, tag="ps")
                    for cd in range(4):
                        nc.tensor.matmul(
                            qps[:],
                            lhsT=w_sb["Wq"][:, cd, ci * 128 : (ci + 1) * 128],
                            rhs=xT_t[:, cd, :],
                            start=(cd == 0),
                            stop=(cd == 3),
                        )
                    e_t = er_pool.tile([128, 512], F32, tag="er")
                    r_t = er_pool.tile([128, 512], F32, tag="er")
                    nc.scalar.activation(e_t[:], qps[:], AF.Exp, bias=bq_sb[:, ci : ci + 1], scale=1.0)
                    nc.scalar.activation(r_t[:], qps[:], AF.Relu, bias=bq_sb[:, ci : ci + 1], scale=1.0)
                    # phi = min(exp(x),1) + relu(x)
                    nc.vector.scalar_tensor_tensor(
                        out=qfT[:, ci, ich * 512 : (ich + 1) * 512],
                        in0=e_t[:],
                        scalar=1.0,
                        in1=r_t[:],
                        op0=ALU.min,
                        op1=ALU.add,
                    )
                # ---- k, v, KV accumulation per s-tile ----
                for it in range(4):
                    ist = ich * 4 + it
                    first, last = (ist == 0), (ist == NT - 1)
                    # k (natural layout) + bias via ones-row matmul
                    kps = ps_a.tile([128, 512], F32, tag="ps")
                    for cd in range(4):
                        nc.tensor.matmul(
                            kps[:],
                            lhsT=xT_t[:, cd, it * 128 : (it + 1) * 128],
                            rhs=w_sb["Wk"][:, cd, :],
                            start=(cd == 0),
                            stop=False,
                        )
                    nc.tensor.matmul(
                        kps[:], lhsT=ones_col[:], rhs=bk_row, start=False, stop=True
                    )
                    e_t = er_pool.tile([128, 512], F32, tag="er")
                    r_t = er_pool.tile([128, 512], F32, tag="er")
                    nc.scalar.activation(e_t[:], kps[:], AF.Exp)
                    nc.scalar.activation(r_t[:], kps[:], AF.Relu)
                    kf = kf_pool.tile([128, 512], DT_MM, tag="kf")
                    nc.vector.scalar_tensor_tensor(
                        out=kf[:], in0=e_t[:], scalar=1.0, in1=r_t[:], op0=ALU.min, op1=ALU.add
                    )
                    # v (natural) with bias fused into the psum->sbuf copy
                    vps = ps_a.tile([128, 512], F32, tag="ps")
                    for cd in range(4):
                        nc.tensor.matmul(
                            vps[:],
                            lhsT=xT_t[:, cd, it * 128 : (it + 1) * 128],
                            rhs=w_sb["Wv"][:, cd, :],
                            start=(cd == 0),
                            stop=(cd == 3),
                        )
                    vq = vqs[ist % 3]
                    nc.vector.tensor_add(
                        out=vq[:, :, 0:128],
                        in0=vps[:].rearrange("p (g n) -> p g n", g=4),
                        in1=bv_rep[:].rearrange("p (g n) -> p g n", g=4),
                    )
                    # KV accumulation per head pair
                    for j in range(4):
                        nc.tensor.matmul(
                            kvq[j],
                            lhsT=kf[:, j * 128 : (j + 1) * 128],
                            rhs=vq[:, j, :],
                            start=first,
                            stop=last,
                        )

            # software pipeline: transposes run one chunk ahead of Q/K/V so the
            # PE has work while weights load and ACT casts the next chunk
            for ich in range(NCH + 1):
                if ich < NCH:
                    transpose_stage(ich)
                if ich == 1:
                    # late-emitted weights: their DVE casts queue behind the
                    # chunk-0/1 xT copies instead of ahead of them
                    emit_weight_cast("Wk")
                    emit_weight("Wv")
                    emit_weight("Wo")
                if ich >= 1:
                    qkv_stage(ich - 1)

            # ---- extract KV blocks and Ksum (still inside pass-A pool scope) ----
            for h in range(H):
                j, rh = h // 2, (h % 2) * 64
                nc.vector.tensor_copy(
                    out=kvsb[rh : rh + 64, j, rh : rh + 64],
                    in_=kvq[j][rh : rh + 64, rh : rh + 64],
                )
            for c in range(4):
                for half in range(2):
                    nc.vector.tensor_copy(
                        out=ksum_bd[half * 64 : (half + 1) * 64, c, 2 * c + half : 2 * c + half + 1],
                        in_=kvq[c][half * 64 : (half + 1) * 64, 128:129],
                    )

        # ======================= PASS B =======================
        # Software pipeline per slice: denT(n+1) leads, OT/out(n) fills the
        # middle, zrep(n+1) trails — by the time the PE reaches zrep(n+1) the
        # ACT Ln/Exp chain for it has long finished, so the PE never stalls
        # on the Z chain.
        with ExitStack() as bctx:
            dz_ps = bctx.enter_context(tc.tile_pool(name="dz_ps", bufs=2, space="PSUM"))
            zrep_ps = bctx.enter_context(tc.tile_pool(name="zrep_ps", bufs=1, space="PSUM"))
            ot_ps = bctx.enter_context(tc.tile_pool(name="ot_ps", bufs=1, space="PSUM"))
            ow_ps = bctx.enter_context(tc.tile_pool(name="ow_ps", bufs=2, space="PSUM"))
            ztsb_pool = bctx.enter_context(tc.tile_pool(name="ztsb_pool", bufs=2))
            zsb_pool = bctx.enter_context(tc.tile_pool(name="zsb_pool", bufs=2))
            otsb_pool = bctx.enter_context(tc.tile_pool(name="otsb_pool", bufs=2))
            out_pool = bctx.enter_context(tc.tile_pool(name="out_pool", bufs=3))

            ztsb_live, zsb_live = {}, {}

            def denT_stage(isl):
                # denT [8, 512] = sum_c ksum_bd[:,c,:]^T @ qfT[:,c,slice] + eps
                dz = dz_ps.tile([8, 512], F32, tag="dz")
                for c in range(4):
                    nc.tensor.matmul(
                        dz[:],
                        lhsT=ksum_bd[:, c, :],
                        rhs=qfT[:, c, isl * 512 : (isl + 1) * 512],
                        start=(c == 0),
                        stop=False,
                    )
                nc.tensor.matmul(dz[:], lhsT=eps_col[:], rhs=ones512[:], start=False, stop=True)
                # Z = 1/den as exp(-ln(den)) on ACT (Reciprocal is banned
                # there; DVE reciprocal is ~6.5ns/elem — too slow at 512 free)
                lntmp = ztsb_pool.tile([8, 512], F32, tag="lntmp")
                nc.scalar.activation(lntmp[:], dz[:], AF.Ln)
                ztsb = ztsb_pool.tile([8, 512], DT_MM, tag="ztsb")
                nc.scalar.activation(ztsb[:], lntmp[:], AF.Exp, scale=-1.0)
                ztsb_live[isl] = ztsb

            def zrep_stage(isl):
                # Zrep: replicate Z rows across head d-partitions (E8 selector
                # matmul), staged to SBUF via ACT (DVE may read only one PSUM
                # operand per op)
                ztsb = ztsb_live.pop(isl)
                zrep_sb = zsb_pool.tile([128, 4, 512], DT_MM, tag="zrep_sb")
                for half in range(2):
                    zrep = zrep_ps.tile([128, 4, 256], F32, tag="zrep")
                    for c in range(4):
                        nc.tensor.matmul(
                            zrep[:, c, :],
                            lhsT=e8[:, c, :],
                            rhs=ztsb[:, half * 256 : (half + 1) * 256],
                            start=True,
                            stop=True,
                        )
                    nc.scalar.activation(
                        zrep_sb[:, :, half * 256 : (half + 1) * 256], zrep[:], AF.Copy
                    )
                zsb_live[isl] = zrep_sb

            def otout_stage(isl):
                zrep_sb = zsb_live.pop(isl)
                for half in range(2):
                    # O^T per head pair (block-diagonal KV)
                    otps = ot_ps.tile([128, 4, 256], F32, tag="ot")
                    for c in range(4):
                        nc.tensor.matmul(
                            otps[:, c, :],
                            lhsT=kvsb[:, c, :],
                            rhs=qfT[:, c, isl * 512 + half * 256 : isl * 512 + (half + 1) * 256],
                            start=True,
                            stop=True,
                        )
                    otsb = otsb_pool.tile([128, 4, 256], DT_MM, tag="otsb")
                    nc.vector.tensor_mul(
                        out=otsb[:],
                        in0=otps[:],
                        in1=zrep_sb[:, :, half * 256 : (half + 1) * 256],
                    )
                    # out = OT^T Wo; +bo fused into the psum->sbuf copy
                    for h2 in range(2):
                        ist = isl * 4 + half * 2 + h2
                        owps = ow_ps.tile([128, 512], F32, tag="ow")
                        for c in range(4):
                            nc.tensor.matmul(
                                owps[:],
                                lhsT=otsb[:, c, h2 * 128 : (h2 + 1) * 128],
                                rhs=w_sb["Wo"][:, c, :],
                                start=(c == 0),
                                stop=(c == 3),
                            )
                        outt = out_pool.tile([128, 512], F32, tag="outt")
                        nc.vector.tensor_add(out=outt[:], in0=owps[:], in1=bo_rep[:])
                        nc.sync.dma_start(out=out_d[ist * 128 : (ist + 1) * 128, :], in_=outt[:])

            denT_stage(0)
            zrep_stage(0)
            for isl in range(NSL):
                if isl + 1 < NSL:
                    denT_stage(isl + 1)
                otout_stage(isl)
                if isl + 1 < NSL:
                    zrep_stage(isl + 1)

    if reps == 1:
        _passes()
    else:
        with tc.For_i(0, reps, 1):
            _passes()


def _legalize_waits(nc: "bass.Bass", max_waits: int = 1) -> int:
    """This toolchain's walrus allows at most ONE sync wait per instruction.

    Tile's scheduler attaches several; hoist the extras into standalone
    event-semaphore (pure wait) instructions on the same engine, placed
    immediately before the original — identical blocking semantics since
    waits execute in stream order on the issuing sequencer.
    """
    n_split = 0
    for func in nc.m.functions:
        for block in func.blocks:
            new_insts = []
            for inst in block.instructions:
                si = getattr(inst, "sync_info", None)
                waits = list(si.on_wait) if (si and si.on_wait) else []
                if len(waits) > max_waits:
                    extra, keep = waits[:-max_waits], waits[-max_waits:]
                    for j, w in enumerate(extra):
                        ev = mybir.InstEventSemaphore(
                            name=f"{inst.name}_lw{j}",
                            engine=inst.engine,
                            ins=[],
                            outs=[],
                            sync_info=mybir.SyncInfo(on_wait=[w], on_update=[]),
                        )
                        new_insts.append(ev)
                        n_split += 1
                    si.on_wait = keep
                new_insts.append(inst)
            block.instructions[:] = new_insts
    return n_split




def build_program(s_total: int = S, reps: int = 1) -> "bass.Bass":
    nc = bass.Bass("TRN2", target_bir_lowering=False, debug=False, num_devices=B)
    io = {
        "x": nc.dram_tensor("x", [s_total, D], F32, kind="ExternalInput").ap(),
        "Wq": nc.dram_tensor("Wq", [D, INNER], F32, kind="ExternalInput").ap(),
        "bq": nc.dram_tensor("bq", [INNER], F32, kind="ExternalInput").ap(),
        "Wk": nc.dram_tensor("Wk", [D, INNER], F32, kind="ExternalInput").ap(),
        "bk": nc.dram_tensor("bk", [INNER], F32, kind="ExternalInput").ap(),
        "Wv": nc.dram_tensor("Wv", [D, INNER], F32, kind="ExternalInput").ap(),
        "bv": nc.dram_tensor("bv", [INNER], F32, kind="ExternalInput").ap(),
        "Wo": nc.dram_tensor("Wo", [INNER, D], F32, kind="ExternalInput").ap(),
        "bo": nc.dram_tensor("bo", [D], F32, kind="ExternalInput").ap(),
        "out": nc.dram_tensor("out", [s_total, D], F32, kind="ExternalOutput").ap(),
    }
    with tile.TileContext(nc) as tc:
        with ExitStack() as ctx:
            _linattn_body(ctx, tc, io, s_total, reps=reps)
    return nc


_PROGRAM_CACHE: dict = {}


def _get_program(s_total: int = S) -> "bass.Bass":
    if s_total not in _PROGRAM_CACHE:
        nc = build_program(s_total)
        _legalize_waits(nc)
        _PROGRAM_CACHE[s_total] = nc
    return _PROGRAM_CACHE[s_total]


def _in_maps(inputs: dict) -> list:
    maps = []
    for b in range(B):
        m = {"x": np.ascontiguousarray(inputs["x"][b], dtype=np.float32)}
        for name in ("Wq", "bq", "Wk", "bk", "Wv", "bv", "Wo", "bo"):
            m[name] = np.ascontiguousarray(inputs[name], dtype=np.float32)
        maps.append(m)
    return maps


def run_hw(inputs: dict, trace: bool = False, **kwargs):
    """Run on the 8 NeuronCores. Returns (out [B,S,D], BassKernelResults)."""
    nc = _get_program(S)
    res = run_bass_kernel_spmd(nc, _in_maps(inputs), list(range(B)), trace=trace, **kwargs)
    out = np.stack([res.results[b]["out"] for b in range(B)], axis=0)
    return out, res


def kernel(**inputs) -> np.ndarray:
    out, _ = run_hw(inputs, trace=False)
    return out


def bench_hw(inputs: dict, iters: int = 20, nc_override=None):
    """Time repeated NEFF executions with device-resident inputs.

    Returns (per_iter_ns, out[B,S,D] from the first run). Uses the same
    shard_map lowering as run_bass_via_pjrt, without donation so input
    buffers can be reused across timed calls.
    """
    import time as _time

    import jax
    from jax.sharding import Mesh, NamedSharding, PartitionSpec
    from jax.experimental.shard_map import shard_map

    from concourse import bass2jax
    from concourse.bass2jax import _bass_exec_p, install_neuronx_cc_hook

    install_neuronx_cc_hook()
    nc = nc_override if nc_override is not None else _get_program(S)
    in_maps = _in_maps(inputs)

    partition_name = nc.partition_id_tensor.name if nc.partition_id_tensor else None
    in_names, out_names, out_avals = [], [], []
    for alloc in nc.m.functions[0].allocations:
        if not isinstance(alloc, mybir.MemoryLocationSet):
            continue
        name = alloc.memorylocations[0].name
        if alloc.kind == "ExternalInput":
            if name != partition_name:
                in_names.append(name)
        elif alloc.kind == "ExternalOutput":
            out_names.append(name)
            out_avals.append(
                jax.core.ShapedArray(tuple(alloc.tensor_shape), mybir.dt.np(alloc.dtype))
            )
    n_params = len(in_names)
    all_in_names = in_names + out_names
    if partition_name is not None:
        all_in_names = all_in_names + [partition_name]

    def _body(*args):
        operands = list(args)
        if partition_name is not None:
            operands.append(bass2jax.partition_id_tensor())
        outs = _bass_exec_p.bind(
            *operands,
            out_avals=tuple(out_avals),
            in_names=tuple(all_in_names),
            out_names=tuple(out_names),
            lowering_input_output_aliases=(),
            sim_require_finite=True,
            sim_require_nnan=True,
            nc=nc,
        )
        return tuple(outs)

    devices = jax.devices()[:B]
    mesh = Mesh(np.asarray(devices), ("core",))
    n_outs = len(out_names)
    in_specs = (PartitionSpec("core"),) * (n_params + n_outs)
    out_specs = (PartitionSpec("core"),) * n_outs
    fn = jax.jit(
        shard_map(_body, mesh=mesh, in_specs=in_specs, out_specs=out_specs, check_rep=False)
    )

    sh = NamedSharding(mesh, PartitionSpec("core"))
    concat_in = [
        jax.device_put(
            np.concatenate([np.asarray(in_maps[c][nm])[None] for c in range(B)], axis=0).reshape(
                B * np.asarray(in_maps[0][nm]).shape[0], *np.asarray(in_maps[0][nm]).shape[1:]
            ),
            sh,
        )
        for nm in in_names
    ]
    concat_zeros = [
        jax.device_put(np.zeros((B * a.shape[0], *a.shape[1:]), a.dtype), sh) for a in out_avals
    ]

    out = fn(*concat_in, *concat_zeros)
    jax.block_until_ready(out)
    first = np.asarray(out[0]).reshape(B, *out_avals[0].shape)

    def timed(f, n):
        t0 = _time.perf_counter()
        for _ in range(n):
            r = f(*concat_in, *concat_zeros)
        jax.block_until_ready(r)
        return (_time.perf_counter() - t0) / n

    timed(fn, 3)
    t = min(timed(fn, max(5, iters // 2)) for _ in range(4))
    return int(t * 1e9), first


def build_copy_program(s_total: int = S) -> "bass.Bass":
    """Same I/O signature as the real program, near-zero work: out = x."""
    nc = bass.Bass("TRN2", target_bir_lowering=False, debug=False, num_devices=B)
    io = {}
    io["x"] = nc.dram_tensor("x", [s_total, D], F32, kind="ExternalInput").ap()
    for nm, shp in (("Wq", [D, INNER]), ("bq", [INNER]), ("Wk", [D, INNER]), ("bk", [INNER]),
                    ("Wv", [D, INNER]), ("bv", [INNER]), ("Wo", [INNER, D]), ("bo", [D])):
        io[nm] = nc.dram_tensor(nm, shp, F32, kind="ExternalInput").ap()
    out_d = nc.dram_tensor("out", [s_total, D], F32, kind="ExternalOutput").ap()
    from contextlib import ExitStack as _ES
    with tile.TileContext(nc) as tc:
        with _ES() as ctx:
            pool = ctx.enter_context(tc.tile_pool(name="cp", bufs=4))
            for i in range(s_total // 128):
                t = pool.tile([128, D], F32, tag="cp")
                sl = slice(i * 128, (i + 1) * 128)
                nc.sync.dma_start(out=t[:], in_=io["x"][sl])
                nc.sync.dma_start(out=out_d[sl], in_=t[:])
    _legalize_waits(nc)
    return nc


# revision 39
# speedup vs baseline: 1.0812x; 1.0206x over previous
"""Trainium2 Bass kernel for LinearAttention (B=8, S=4096, D=512, H=8, DH=64).

Sharding: data-parallel over batch — core b processes batch element b end-to-end.

Per-core pipeline (matmul inputs in bf16; PSUM accumulation in f32):
  pass A (per 512-wide s-chunk):
    x chunk -> one 1MB DMA -> ACT cast to bf16 -> PE-transpose (bf16, 1 cyc/row)
    qT = Wq^T x^T (psum), phi -> QfT [inner, s]  (bias via ACT per-partition bias)
    k  = x Wk + bk (rank-1 ones-row matmul), phi -> Kf [s, inner]
    v  = x Wv; +bv fused in the psum->sbuf copy -> v' [128, pair, 128+ones]
    KV accumulated per head pair in [128, 129] psum tiles (col 128 = Ksum)
  pass B (per 512-wide s-slice):
    denT = Ksum_bd^T Qf^T in [8, s] layout (block-diag lhsT) + eps rank-1 matmul
    Z^T = 1/denT via DVE recip -> bf16
    Zrep via selector matmul (E8), kept in PSUM
    O^T = KV^T @ QfT per head pair (block-diagonal kvsb)
    OT = O^T * Zrep (DVE, both operands read from PSUM) -> bf16
    out = OT^T Wo + bo (rank-1 matmul into same psum) -> DMA PSUM -> HBM direct
"""

import os
import sys

import numpy as np

for _p in ("/opt/trn_rl_repo",):
    if os.path.isdir(_p) and _p not in sys.path:
        sys.path.insert(0, _p)

from contextlib import ExitStack

import concourse.bass as bass
import concourse.mybir as mybir
import concourse.tile as tile
from concourse.bass_utils import run_bass_kernel_spmd
from concourse.masks import make_identity
from concourse import library_config

B, S, D = 8, 4096, 512
H, DH = 8, 64
INNER = H * DH  # 512
EPS = 1e-6

F32 = mybir.dt.float32
F32R = mybir.dt.float32r
BF16 = mybir.dt.bfloat16
AF = mybir.ActivationFunctionType
ALU = mybir.AluOpType

# matmul input dtype: "bf16" (full-rate, ~5e-3 max-rel) or "f32r" (full-rate for
# moving dims >= 256, ~3e-4) — tolerance gate is 2e-2.
MM_DTYPE = os.environ.get("LINATTN_MM_DTYPE", "bf16")
DT_MM = {"bf16": BF16, "f32r": F32R, "f32": F32}[MM_DTYPE]


def _linattn_body(ctx: ExitStack, tc: "tile.TileContext", io: dict, s_total: int, reps: int = 1):
    nc = tc.nc
    NT = s_total // 128  # s-tiles
    NCH = s_total // 512  # pass-A chunks
    NSL = s_total // 512  # pass-B slices (512-wide)

    x_d = io["x"]
    out_d = io["out"]

    singles = ctx.enter_context(tc.tile_pool(name="singles", bufs=1))

    # ---- constants first (gpsimd): ident needed by the very first transposes ----
    ident = singles.tile([128, 128], DT_MM, name="ident", tag="ident")
    make_identity(nc, ident[:])

    # ---- weights: [128, 4, 512] in DT_MM; chunk c holds rows c*128..(c+1)*128 ----
    # One 1MB DMA per weight on the scalar queue (sync queue is reserved for x /
    # out traffic so x chunk 0 starts at t=0); DVE copies round into DT_MM.
    # NOTE: staging pools stay open for the whole kernel — closing them lets
    # pass-A pools reuse their SBUF, and the resulting WAR dependencies
    # serialize the first x DMA behind the last weight cast (~25us stall).
    # Wv/Wo are emitted later (inside the pass-A pipeline) so their DVE casts
    # don't sit ahead of the chunk-0 xT copies in the DVE queue.
    w_sb = {}
    wraw_pool = ctx.enter_context(tc.tile_pool(name="wraw_pool", bufs=2))

    def emit_weight(name):
        raw = wraw_pool.tile([128, 4, INNER], F32, tag="wraw")
        nc.scalar.dma_start(out=raw[:], in_=io[name].rearrange("(c p) n -> p c n", p=128))
        t = singles.tile([128, 4, INNER], DT_MM, name=f"{name}_sb", tag=f"{name}_sb")
        for c in range(4):
            nc.vector.tensor_copy(out=t[:, c, :], in_=raw[:, c, :])
        w_sb[name] = t

    emit_weight("Wq")
    emit_weight("Wk")
    # bias rows for rank-1 PE bias adds (both on partition 0)
    brow_raw = wraw_pool.tile([1, 2, INNER], F32, tag="brow_raw")
    nc.gpsimd.dma_start(out=brow_raw[:, 0, :], in_=io["bk"].rearrange("(a n) -> a n", a=1))
    nc.gpsimd.dma_start(out=brow_raw[:, 1, :], in_=io["bo"].rearrange("(a n) -> a n", a=1))
    brow = singles.tile([1, 2, INNER], DT_MM, name="brow", tag="brow")
    nc.vector.tensor_copy(out=brow[:], in_=brow_raw[:])
    bk_row, bo_row = brow[:, 0, :], brow[:, 1, :]

    # ---- biases ----
    bq_sb = singles.tile([128, 4], F32, name="bq_sb", tag="bq_sb")
    nc.gpsimd.dma_start(out=bq_sb[:], in_=io["bq"].rearrange("(c p) -> p c", p=128))
    # bv/bo replicated across partitions via partition-step-0 DMA (DVE add operands)
    rep = {}
    for nm in ("bv", "bo"):
        t = singles.tile([128, INNER], F32, name=f"{nm}_rep", tag=f"{nm}_rep")
        ap = io[nm]
        nc.gpsimd.dma_start(
            out=t[:],
            in_=bass.AP(tensor=ap.tensor, offset=ap.offset, ap=[[0, 128]] + list(ap.ap)),
        )
        rep[nm] = t
    bv_rep, bo_rep = rep["bv"], rep["bo"]

    # ---- more constants ----
    # GPSIMD memset/affine_select cannot write f32r: stage in F32, DVE-copy over.
    ones_vcol = singles.tile([128, 4, 1], F32, name="ones_vcol", tag="ones_vcol")
    ones_col = singles.tile([1, 128], DT_MM, name="ones_col", tag="ones_col")
    ones512 = singles.tile([1, 512], DT_MM, name="ones512", tag="ones512")
    eps_col = singles.tile([1, 8], DT_MM, name="eps_col", tag="eps_col")
    e8 = singles.tile([8, 4, 128], DT_MM, name="e8", tag="e8")
    if True:
        cst = ctx.enter_context(tc.tile_pool(name="const_stage", bufs=4))
        nc.gpsimd.memset(ones_vcol[:], 1.0)
        st1 = cst.tile([1, 512], F32, tag="st1")
        nc.gpsimd.memset(st1[:], 1.0)
        nc.vector.tensor_copy(out=ones512[:], in_=st1[:])
        nc.vector.tensor_copy(out=ones_col[:], in_=st1[:, 0:128])
        ste = cst.tile([1, 8], F32, tag="ste")
        nc.gpsimd.memset(ste[:], EPS)
        nc.vector.tensor_copy(out=eps_col[:], in_=ste[:])
        st8 = cst.tile([8, 4, 128], F32, tag="st8")
        nc.gpsimd.memset(st8[:], 0.0)
        nc.gpsimd.affine_select(
            out=st8[:, :, 0:64], in_=st8[:, :, 0:64], compare_op=ALU.not_equal, fill=1.0,
            base=0, pattern=[[-2, 4], [0, 64]], channel_multiplier=1,
        )
        nc.gpsimd.affine_select(
            out=st8[:, :, 64:128], in_=st8[:, :, 64:128], compare_op=ALU.not_equal, fill=1.0,
            base=-1, pattern=[[-2, 4], [0, 64]], channel_multiplier=1,
        )
        nc.vector.tensor_copy(out=e8[:], in_=st8[:])

    # ---- persistent per-core buffers ----
    qfT = singles.tile([128, 4, s_total], DT_MM, name="qfT", tag="qfT")  # [inner, s]
    kvsb = singles.tile([128, 4, 128], DT_MM, name="kvsb", tag="kvsb")  # block-diag per pair
    # v' staging buffers (manual 3-deep rotation so the ones column is written once)
    vqs = [singles.tile([128, 4, 129], DT_MM, name=f"vq{i}", tag=f"vq{i}") for i in range(3)]
    for i in range(3):
        nc.vector.tensor_copy(out=vqs[i][:, :, 128:129], in_=ones_vcol[:])
    # block-diag Ksum rhs: [128, pair, 8]; pair c: rows 0-63 -> col 2c, rows 64-127 -> col 2c+1
    ksum_bd = singles.tile([128, 4, 8], DT_MM, name="ksum_bd", tag="ksum_bd")
    if True:
        kbz = ctx.enter_context(tc.tile_pool(name="kbz_stage", bufs=2))
        stz = kbz.tile([128, 4, 8], F32, tag="stz")
        nc.gpsimd.memset(stz[:], 0.0)
        nc.vector.tensor_copy(out=ksum_bd[:], in_=stz[:])
        stz2 = kbz.tile([128, 4, 128], F32, tag="stz2")
        nc.gpsimd.memset(stz2[:], 0.0)
        nc.vector.tensor_copy(out=kvsb[:], in_=stz2[:])

    def _passes():
        # =================== PASS A ===================
        with ExitStack() as actx:
            x_pool = actx.enter_context(tc.tile_pool(name="x_pool", bufs=2))
            xbf_pool = actx.enter_context(tc.tile_pool(name="xbf_pool", bufs=3))
            xT_pool = actx.enter_context(tc.tile_pool(name="xT_pool", bufs=2))
            er_pool = actx.enter_context(tc.tile_pool(name="er_pool", bufs=6))
            kf_pool = actx.enter_context(tc.tile_pool(name="kf_pool", bufs=3))
            v_pool = actx.enter_context(tc.tile_pool(name="v_pool", bufs=3))
            ps_a = actx.enter_context(tc.tile_pool(name="ps_a", bufs=4, space="PSUM"))
            ps_acc = actx.enter_context(tc.tile_pool(name="ps_acc", bufs=1, space="PSUM"))

            # KV accumulators per head pair (one PSUM bank each — interleaved
            # accumulation groups must not share a bank):
            # kvq[j] = cols 0-127 pair j's v cols, col 128 = Ksum
            kvq = [
                ps_acc.tile([128, 129], F32, name=f"kvq_{j}", tag=f"kvq_{j}")[:]
                for j in range(4)
            ]

            xT_live = {}

            def transpose_stage(ich):
                # ---- one 1MB DMA for the whole 512-row chunk ----
                xc = x_pool.tile([128, 4, D], F32, tag="x")
                nc.sync.dma_start(
                    out=xc[:],
                    in_=x_d[ich * 512 : (ich + 1) * 512, :].rearrange("(t p) d -> p t d", p=128),
                )
                xT_t = xT_pool.tile([128, 4, 512], DT_MM, tag="xT")
                # ---- cast to bf16 on ACT, transpose chunk on PE (1 cyc/row) ----
                for it in range(4):
                    xbf = xbf_pool.tile([128, D], DT_MM, tag="xbf")
                    nc.scalar.activation(xbf[:], xc[:, it, :], AF.Copy)
                    xps = ps_a.tile([128, 4, 128], DT_MM, tag="ps")
                    for c in range(4):
                        nc.tensor.transpose(xps[:, c, :], xbf[:, c * 128 : (c + 1) * 128], ident[:])
                    nc.vector.tensor_copy(out=xT_t[:, :, it * 128 : (it + 1) * 128], in_=xps[:])
                xT_live[ich] = xT_t

            def qkv_stage(ich):
                xT_t = xT_live.pop(ich)
                # ---- qT = Wq^T xT, phi -> QfT ----
                for ci in range(4):
                    qps = ps_a.tile([128, 512], F32, tag="ps")
                    for cd in range(4):
                        nc.tensor.matmul(
                            qps[:],
                            lhsT=w_sb["Wq"][:, cd, ci * 128 : (ci + 1) * 128],
                            rhs=xT_t[:, cd, :],
                            start=(cd == 0),
                            stop=(cd == 3),
                        )
                    e_t = er_pool.tile([128, 512], F32, tag="er")
                    r_t = er_pool.tile([128, 512], F32, tag="er")
                    nc.scalar.activation(e_t[:], qps[:], AF.Exp, bias=bq_sb[:, ci : ci + 1], scale=1.0)
                    nc.scalar.activation(r_t[:], qps[:], AF.Relu, bias=bq_sb[:, ci : ci + 1], scale=1.0)
                    # phi = min(exp(x),1) + relu(x)
                    nc.vector.scalar_tensor_tensor(
                        out=qfT[:, ci, ich * 512 : (ich + 1) * 512],
                        in0=e_t[:],
                        scalar=1.0,
                        in1=r_t[:],
                        op0=ALU.min,
                        op1=ALU.add,
                    )
                # ---- k, v, KV accumulation per s-tile ----
                for it in range(4):
                    ist = ich * 4 + it
                    first, last = (ist == 0), (ist == NT - 1)
                    # k (natural layout) + bias via ones-row matmul
                    kps = ps_a.tile([128, 512], F32, tag="ps")
                    for cd in range(4):
                        nc.tensor.matmul(
                            kps[:],
                            lhsT=xT_t[:, cd, it * 128 : (it + 1) * 128],
                            rhs=w_sb["Wk"][:, cd, :],
                            start=(cd == 0),
                            stop=False,
                        )
                    nc.tensor.matmul(
                        kps[:], lhsT=ones_col[:], rhs=bk_row, start=False, stop=True
                    )
                    e_t = er_pool.tile([128, 512], F32, tag="er")
                    r_t = er_pool.tile([128, 512], F32, tag="er")
                    nc.scalar.activation(e_t[:], kps[:], AF.Exp)
                    nc.scalar.activation(r_t[:], kps[:], AF.Relu)
                    kf = kf_pool.tile([128, 512], DT_MM, tag="kf")
                    nc.vector.scalar_tensor_tensor(
                        out=kf[:], in0=e_t[:], scalar=1.0, in1=r_t[:], op0=ALU.min, op1=ALU.add
                    )
                    # v (natural) with bias fused into the psum->sbuf copy
                    vps = ps_a.tile([128, 512], F32, tag="ps")
                    for cd in range(4):
                        nc.tensor.matmul(
                            vps[:],
                            lhsT=xT_t[:, cd, it * 128 : (it + 1) * 128],
                            rhs=w_sb["Wv"][:, cd, :],
                            start=(cd == 0),
                            stop=(cd == 3),
                        )
                    vq = vqs[ist % 3]
                    nc.vector.tensor_add(
                        out=vq[:, :, 0:128],
                        in0=vps[:].rearrange("p (g n) -> p g n", g=4),
                        in1=bv_rep[:].rearrange("p (g n) -> p g n", g=4),
                    )
                    # KV accumulation per head pair
                    for j in range(4):
                        nc.tensor.matmul(
                            kvq[j],
                            lhsT=kf[:, j * 128 : (j + 1) * 128],
                            rhs=vq[:, j, :],
                            start=first,
                            stop=last,
                        )

            # software pipeline: transposes run one chunk ahead of Q/K/V so the
            # PE has work while weights load and ACT casts the next chunk
            for ich in range(NCH + 1):
                if ich < NCH:
                    transpose_stage(ich)
                if ich == 0:
                    # late-emitted weights: their DVE casts queue behind the
                    # chunk-0 xT copies instead of ahead of them
                    emit_weight("Wv")
                    emit_weight("Wo")
                if ich >= 1:
                    qkv_stage(ich - 1)

            # ---- extract KV blocks and Ksum (still inside pass-A pool scope) ----
            for h in range(H):
                j, rh = h // 2, (h % 2) * 64
                nc.vector.tensor_copy(
                    out=kvsb[rh : rh + 64, j, rh : rh + 64],
                    in_=kvq[j][rh : rh + 64, rh : rh + 64],
                )
            for c in range(4):
                for half in range(2):
                    nc.vector.tensor_copy(
                        out=ksum_bd[half * 64 : (half + 1) * 64, c, 2 * c + half : 2 * c + half + 1],
                        in_=kvq[c][half * 64 : (half + 1) * 64, 128:129],
                    )

        # ======================= PASS B =======================
        with ExitStack() as bctx:
            dz_ps = bctx.enter_context(tc.tile_pool(name="dz_ps", bufs=2, space="PSUM"))
            zrep_ps = bctx.enter_context(tc.tile_pool(name="zrep_ps", bufs=1, space="PSUM"))
            ot_ps = bctx.enter_context(tc.tile_pool(name="ot_ps", bufs=1, space="PSUM"))
            ow_ps = bctx.enter_context(tc.tile_pool(name="ow_ps", bufs=2, space="PSUM"))
            ztsb_pool = bctx.enter_context(tc.tile_pool(name="ztsb_pool", bufs=2))
            otsb_pool = bctx.enter_context(tc.tile_pool(name="otsb_pool", bufs=2))
            out_pool = bctx.enter_context(tc.tile_pool(name="out_pool", bufs=3))

            for isl in range(NSL):
                # denT [8, 512] = sum_c ksum_bd[:,c,:]^T @ qfT[:,c,slice] + eps
                dz = dz_ps.tile([8, 512], F32, tag="dz")
                for c in range(4):
                    nc.tensor.matmul(
                        dz[:],
                        lhsT=ksum_bd[:, c, :],
                        rhs=qfT[:, c, isl * 512 : (isl + 1) * 512],
                        start=(c == 0),
                        stop=False,
                    )
                nc.tensor.matmul(dz[:], lhsT=eps_col[:], rhs=ones512[:], start=False, stop=True)
                # Z = 1/den as exp(-ln(den)) on ACT (Reciprocal is banned there,
                # and DVE reciprocal costs ~6.5ns/elem — too slow at 512 free)
                lntmp = ztsb_pool.tile([8, 512], F32, tag="lntmp")
                nc.scalar.activation(lntmp[:], dz[:], AF.Ln)
                ztsb = ztsb_pool.tile([8, 512], DT_MM, tag="ztsb")
                nc.scalar.activation(ztsb[:], lntmp[:], AF.Exp, scale=-1.0)

                for half in range(2):
                    # O^T per head pair (block-diagonal KV) — emitted before zrep
                    # so the PE streams OT while ACT finishes the Z chain
                    otps = ot_ps.tile([128, 4, 256], F32, tag="ot")
                    for c in range(4):
                        nc.tensor.matmul(
                            otps[:, c, :],
                            lhsT=kvsb[:, c, :],
                            rhs=qfT[:, c, isl * 512 + half * 256 : isl * 512 + (half + 1) * 256],
                            start=True,
                            stop=True,
                        )
                    # Zrep: replicate Z rows across head d-partitions (E8 selector matmul)
                    zrep = zrep_ps.tile([128, 4, 256], F32, tag="zrep")
                    for c in range(4):
                        nc.tensor.matmul(
                            zrep[:, c, :],
                            lhsT=e8[:, c, :],
                            rhs=ztsb[:, half * 256 : (half + 1) * 256],
                            start=True,
                            stop=True,
                        )
                    # DVE may read only one PSUM operand per op: stage zrep via ACT
                    zrep_sb = otsb_pool.tile([128, 4, 256], DT_MM, tag="zrep_sb")
                    nc.scalar.activation(zrep_sb[:], zrep[:], AF.Copy)
                    otsb = otsb_pool.tile([128, 4, 256], DT_MM, tag="otsb")
                    nc.vector.tensor_mul(out=otsb[:], in0=otps[:], in1=zrep_sb[:])
                    # out = OT^T Wo; +bo fused into the psum->sbuf copy
                    for h2 in range(2):
                        ist = isl * 4 + half * 2 + h2
                        owps = ow_ps.tile([128, 512], F32, tag="ow")
                        for c in range(4):
                            nc.tensor.matmul(
                                owps[:],
                                lhsT=otsb[:, c, h2 * 128 : (h2 + 1) * 128],
                                rhs=w_sb["Wo"][:, c, :],
                                start=(c == 0),
                                stop=(c == 3),
                            )
                        outt = out_pool.tile([128, 512], F32, tag="outt")
                        nc.vector.tensor_add(out=outt[:], in0=owps[:], in1=bo_rep[:])
                        nc.sync.dma_start(out=out_d[ist * 128 : (ist + 1) * 128, :], in_=outt[:])

    if reps == 1:
        _passes()
    else:
        with tc.For_i(0, reps, 1):
            _passes()


def _legalize_waits(nc: "bass.Bass", max_waits: int = 1) -> int:
    """This toolchain's walrus allows at most ONE sync wait per instruction.

    Tile's scheduler attaches several; hoist the extras into standalone
    event-semaphore (pure wait) instructions on the same engine, placed
    immediately before the original — identical blocking semantics since
    waits execute in stream order on the issuing sequencer.
    """
    n_split = 0
    for func in nc.m.functions:
        for block in func.blocks:
            new_insts = []
            for inst in block.instructions:
                si = getattr(inst, "sync_info", None)
                waits = list(si.on_wait) if (si and si.on_wait) else []
                if len(waits) > max_waits:
                    extra, keep = waits[:-max_waits], waits[-max_waits:]
                    for j, w in enumerate(extra):
                        ev = mybir.InstEventSemaphore(
                            name=f"{inst.name}_lw{j}",
                            engine=inst.engine,
                            ins=[],
                            outs=[],
                            sync_info=mybir.SyncInfo(on_wait=[w], on_update=[]),
                        )
                        new_insts.append(ev)
                        n_split += 1
                    si.on_wait = keep
                new_insts.append(inst)
            block.instructions[:] = new_insts
    return n_split




def build_program(s_total: int = S, reps: int = 1) -> "bass.Bass":
    nc = bass.Bass("TRN2", target_bir_lowering=False, debug=False, num_devices=B)
    io = {
        "x": nc.dram_tensor("x", [s_total, D], F32, kind="ExternalInput").ap(),
        "Wq": nc.dram_tensor("Wq", [D, INNER], F32, kind="ExternalInput").ap(),
        "bq": nc.dram_tensor("bq", [INNER], F32, kind="ExternalInput").ap(),
        "Wk": nc.dram_tensor("Wk", [D, INNER], F32, kind="ExternalInput").ap(),
        "bk": nc.dram_tensor("bk", [INNER], F32, kind="ExternalInput").ap(),
        "Wv": nc.dram_tensor("Wv", [D, INNER], F32, kind="ExternalInput").ap(),
        "bv": nc.dram_tensor("bv", [INNER], F32, kind="ExternalInput").ap(),
        "Wo": nc.dram_tensor("Wo", [INNER, D], F32, kind="ExternalInput").ap(),
        "bo": nc.dram_tensor("bo", [D], F32, kind="ExternalInput").ap(),
        "out": nc.dram_tensor("out", [s_total, D], F32, kind="ExternalOutput").ap(),
    }
    with tile.TileContext(nc) as tc:
        with ExitStack() as ctx:
            _linattn_body(ctx, tc, io, s_total, reps=reps)
    return nc


_PROGRAM_CACHE: dict = {}


def _get_program(s_total: int = S) -> "bass.Bass":
    if s_total not in _PROGRAM_CACHE:
        nc = build_program(s_total)
        _legalize_waits(nc)
        _PROGRAM_CACHE[s_total] = nc
    return _PROGRAM_CACHE[s_total]


def _in_maps(inputs: dict) -> list:
    maps = []
    for b in range(B):
        m = {"x": np.ascontiguousarray(inputs["x"][b], dtype=np.float32)}
        for name in ("Wq", "bq", "Wk", "bk", "Wv", "bv", "Wo", "bo"):
            m[name] = np.ascontiguousarray(inputs[name], dtype=np.float32)
        maps.append(m)
    return maps


def run_hw(inputs: dict, trace: bool = False, **kwargs):
    """Run on the 8 NeuronCores. Returns (out [B,S,D], BassKernelResults)."""
    nc = _get_program(S)
    res = run_bass_kernel_spmd(nc, _in_maps(inputs), list(range(B)), trace=trace, **kwargs)
    out = np.stack([res.results[b]["out"] for b in range(B)], axis=0)
    return out, res


def kernel(**inputs) -> np.ndarray:
    out, _ = run_hw(inputs, trace=False)
    return out


def bench_hw(inputs: dict, iters: int = 20, nc_override=None):
    """Time repeated NEFF executions with device-resident inputs.

    Returns (per_iter_ns, out[B,S,D] from the first run). Uses the same
    shard_map lowering as run_bass_via_pjrt, without donation so input
    buffers can be reused across timed calls.
    """
    import time as _time

    import jax
    from jax.sharding import Mesh, NamedSharding, PartitionSpec
    from jax.experimental.shard_map import shard_map

    from concourse import bass2jax
    from concourse.bass2jax import _bass_exec_p, install_neuronx_cc_hook

    install_neuronx_cc_hook()
    nc = nc_override if nc_override is not None else _get_program(S)
    in_maps = _in_maps(inputs)

    partition_name = nc.partition_id_tensor.name if nc.partition_id_tensor else None
    in_names, out_names, out_avals = [], [], []
    for alloc in nc.m.functions[0].allocations:
        if not isinstance(alloc, mybir.MemoryLocationSet):
            continue
        name = alloc.memorylocations[0].name
        if alloc.kind == "ExternalInput":
            if name != partition_name:
                in_names.append(name)
        elif alloc.kind == "ExternalOutput":
            out_names.append(name)
            out_avals.append(
                jax.core.ShapedArray(tuple(alloc.tensor_shape), mybir.dt.np(alloc.dtype))
            )
    n_params = len(in_names)
    all_in_names = in_names + out_names
    if partition_name is not None:
        all_in_names = all_in_names + [partition_name]

    def _body(*args):
        operands = list(args)
        if partition_name is not None:
            operands.append(bass2jax.partition_id_tensor())
        outs = _bass_exec_p.bind(
            *operands,
            out_avals=tuple(out_avals),
            in_names=tuple(all_in_names),
            out_names=tuple(out_names),
            lowering_input_output_aliases=(),
            sim_require_finite=True,
            sim_require_nnan=True,
            nc=nc,
        )
        return tuple(outs)

    devices = jax.devices()[:B]
    mesh = Mesh(np.asarray(devices), ("core",))
    n_outs = len(out_names)
    in_specs = (PartitionSpec("core"),) * (n_params + n_outs)
    out_specs = (PartitionSpec("core"),) * n_outs
    fn = jax.jit(
        shard_map(_body, mesh=mesh, in_specs=in_specs, out_specs=out_specs, check_rep=False)
    )

    sh = NamedSharding(mesh, PartitionSpec("core"))
    concat_in = [
        jax.device_put(
            np.concatenate([np.asarray(in_maps[c][nm])[None] for c in range(B)], axis=0).reshape(
                B * np.asarray(in_maps[0][nm]).shape[0], *np.asarray(in_maps[0][nm]).shape[1:]
            ),
            sh,
        )
        for nm in in_names
    ]
    concat_zeros = [
        jax.device_put(np.zeros((B * a.shape[0], *a.shape[1:]), a.dtype), sh) for a in out_avals
    ]

    out = fn(*concat_in, *concat_zeros)
    jax.block_until_ready(out)
    first = np.asarray(out[0]).reshape(B, *out_avals[0].shape)

    def timed(f, n):
        t0 = _time.perf_counter()
        for _ in range(n):
            r = f(*concat_in, *concat_zeros)
        jax.block_until_ready(r)
        return (_time.perf_counter() - t0) / n

    timed(fn, 3)
    t = min(timed(fn, max(5, iters // 2)) for _ in range(4))
    return int(t * 1e9), first


def build_copy_program(s_total: int = S) -> "bass.Bass":
    """Same I/O signature as the real program, near-zero work: out = x."""
    nc = bass.Bass("TRN2", target_bir_lowering=False, debug=False, num_devices=B)
    io = {}
    io["x"] = nc.dram_tensor("x", [s_total, D], F32, kind="ExternalInput").ap()
    for nm, shp in (("Wq", [D, INNER]), ("bq", [INNER]), ("Wk", [D, INNER]), ("bk", [INNER]),
                    ("Wv", [D, INNER]), ("bv", [INNER]), ("Wo", [INNER, D]), ("bo", [D])):
        io[nm] = nc.dram_tensor(nm, shp, F32, kind="ExternalInput").ap()
    out_d = nc.dram_tensor("out", [s_total, D], F32, kind="ExternalOutput").ap()
    from contextlib import ExitStack as _ES
    with tile.TileContext(nc) as tc:
        with _ES() as ctx:
            pool = ctx.enter_context(tc.tile_pool(name="cp", bufs=4))
            for i in range(s_total // 128):
                t = pool.tile([128, D], F32, tag="cp")
                sl = slice(i * 128, (i + 1) * 128)
                nc.sync.dma_start(out=t[:], in_=io["x"][sl])
                nc.sync.dma_start(out=out_d[sl], in_=t[:])
    _legalize_waits(nc)
    return nc


# revision 40
# speedup vs baseline: 1.0889x; 1.0071x over previous
"""Trainium2 Bass kernel for LinearAttention (B=8, S=4096, D=512, H=8, DH=64).

Sharding: data-parallel over batch — core b processes batch element b end-to-end.

Per-core pipeline (matmul inputs in bf16; PSUM accumulation in f32):
  pass A (per 512-wide s-chunk):
    x chunk -> one 1MB DMA -> ACT cast to bf16 -> PE-transpose (bf16, 1 cyc/row)
    qT = Wq^T x^T (psum), phi -> QfT [inner, s]  (bias via ACT per-partition bias)
    k  = x Wk + bk (rank-1 ones-row matmul), phi -> Kf [s, inner]
    v  = x Wv; +bv fused in the psum->sbuf copy -> v' [128, pair, 128+ones]
    KV accumulated per head pair in [128, 129] psum tiles (col 128 = Ksum)
  pass B (per 512-wide s-slice):
    denT = Ksum_bd^T Qf^T in [8, s] layout (block-diag lhsT) + eps rank-1 matmul
    Z^T = 1/denT via DVE recip -> bf16
    Zrep via selector matmul (E8), kept in PSUM
    O^T = KV^T @ QfT per head pair (block-diagonal kvsb)
    OT = O^T * Zrep (DVE, both operands read from PSUM) -> bf16
    out = OT^T Wo + bo (rank-1 matmul into same psum) -> DMA PSUM -> HBM direct
"""

import os
import sys

import numpy as np

for _p in ("/opt/trn_rl_repo",):
    if os.path.isdir(_p) and _p not in sys.path:
        sys.path.insert(0, _p)

from contextlib import ExitStack

import concourse.bass as bass
import concourse.mybir as mybir
import concourse.tile as tile
from concourse.bass_utils import run_bass_kernel_spmd
from concourse.masks import make_identity
from concourse import library_config

B, S, D = 8, 4096, 512
H, DH = 8, 64
INNER = H * DH  # 512
EPS = 1e-6

F32 = mybir.dt.float32
F32R = mybir.dt.float32r
BF16 = mybir.dt.bfloat16
AF = mybir.ActivationFunctionType
ALU = mybir.AluOpType

# matmul input dtype: "bf16" (full-rate, ~5e-3 max-rel) or "f32r" (full-rate for
# moving dims >= 256, ~3e-4) — tolerance gate is 2e-2.
MM_DTYPE = os.environ.get("LINATTN_MM_DTYPE", "bf16")
DT_MM = {"bf16": BF16, "f32r": F32R, "f32": F32}[MM_DTYPE]


def _linattn_body(ctx: ExitStack, tc: "tile.TileContext", io: dict, s_total: int, reps: int = 1):
    nc = tc.nc
    NT = s_total // 128  # s-tiles
    NCH = s_total // 512  # pass-A chunks
    NSL = s_total // 512  # pass-B slices (512-wide)

    x_d = io["x"]
    out_d = io["out"]

    singles = ctx.enter_context(tc.tile_pool(name="singles", bufs=1))

    # ---- constants first (gpsimd): ident needed by the very first transposes ----
    ident = singles.tile([128, 128], DT_MM, name="ident", tag="ident")
    make_identity(nc, ident[:])

    # ---- weights: [128, 4, 512] in DT_MM; chunk c holds rows c*128..(c+1)*128 ----
    # One 1MB DMA per weight on the scalar queue (sync queue is reserved for x /
    # out traffic so x chunk 0 starts at t=0); DVE copies round into DT_MM.
    # NOTE: staging pools stay open for the whole kernel — closing them lets
    # pass-A pools reuse their SBUF, and the resulting WAR dependencies
    # serialize the first x DMA behind the last weight cast (~25us stall).
    # Wv/Wo are emitted later (inside the pass-A pipeline) so their DVE casts
    # don't sit ahead of the chunk-0 xT copies in the DVE queue.
    w_sb = {}
    wraw_pool = ctx.enter_context(tc.tile_pool(name="wraw_pool", bufs=2))

    def emit_weight(name):
        raw = wraw_pool.tile([128, 4, INNER], F32, tag="wraw")
        nc.scalar.dma_start(out=raw[:], in_=io[name].rearrange("(c p) n -> p c n", p=128))
        t = singles.tile([128, 4, INNER], DT_MM, name=f"{name}_sb", tag=f"{name}_sb")
        for c in range(4):
            nc.vector.tensor_copy(out=t[:, c, :], in_=raw[:, c, :])
        w_sb[name] = t

    emit_weight("Wq")
    emit_weight("Wk")
    # bias rows for rank-1 PE bias adds (both on partition 0)
    brow_raw = wraw_pool.tile([1, 2, INNER], F32, tag="brow_raw")
    nc.gpsimd.dma_start(out=brow_raw[:, 0, :], in_=io["bk"].rearrange("(a n) -> a n", a=1))
    nc.gpsimd.dma_start(out=brow_raw[:, 1, :], in_=io["bo"].rearrange("(a n) -> a n", a=1))
    brow = singles.tile([1, 2, INNER], DT_MM, name="brow", tag="brow")
    nc.vector.tensor_copy(out=brow[:], in_=brow_raw[:])
    bk_row, bo_row = brow[:, 0, :], brow[:, 1, :]

    # ---- biases ----
    bq_sb = singles.tile([128, 4], F32, name="bq_sb", tag="bq_sb")
    nc.gpsimd.dma_start(out=bq_sb[:], in_=io["bq"].rearrange("(c p) -> p c", p=128))
    # bv/bo replicated across partitions via partition-step-0 DMA (DVE add operands)
    rep = {}
    for nm in ("bv", "bo"):
        t = singles.tile([128, INNER], F32, name=f"{nm}_rep", tag=f"{nm}_rep")
        ap = io[nm]
        nc.gpsimd.dma_start(
            out=t[:],
            in_=bass.AP(tensor=ap.tensor, offset=ap.offset, ap=[[0, 128]] + list(ap.ap)),
        )
        rep[nm] = t
    bv_rep, bo_rep = rep["bv"], rep["bo"]

    # ---- more constants ----
    # GPSIMD memset/affine_select cannot write f32r: stage in F32, DVE-copy over.
    ones_vcol = singles.tile([128, 4, 1], F32, name="ones_vcol", tag="ones_vcol")
    ones_col = singles.tile([1, 128], DT_MM, name="ones_col", tag="ones_col")
    ones512 = singles.tile([1, 512], DT_MM, name="ones512", tag="ones512")
    eps_col = singles.tile([1, 8], DT_MM, name="eps_col", tag="eps_col")
    e8 = singles.tile([8, 4, 128], DT_MM, name="e8", tag="e8")
    if True:
        cst = ctx.enter_context(tc.tile_pool(name="const_stage", bufs=4))
        nc.gpsimd.memset(ones_vcol[:], 1.0)
        st1 = cst.tile([1, 512], F32, tag="st1")
        nc.gpsimd.memset(st1[:], 1.0)
        nc.vector.tensor_copy(out=ones512[:], in_=st1[:])
        nc.vector.tensor_copy(out=ones_col[:], in_=st1[:, 0:128])
        ste = cst.tile([1, 8], F32, tag="ste")
        nc.gpsimd.memset(ste[:], EPS)
        nc.vector.tensor_copy(out=eps_col[:], in_=ste[:])
        st8 = cst.tile([8, 4, 128], F32, tag="st8")
        nc.gpsimd.memset(st8[:], 0.0)
        nc.gpsimd.affine_select(
            out=st8[:, :, 0:64], in_=st8[:, :, 0:64], compare_op=ALU.not_equal, fill=1.0,
            base=0, pattern=[[-2, 4], [0, 64]], channel_multiplier=1,
        )
        nc.gpsimd.affine_select(
            out=st8[:, :, 64:128], in_=st8[:, :, 64:128], compare_op=ALU.not_equal, fill=1.0,
            base=-1, pattern=[[-2, 4], [0, 64]], channel_multiplier=1,
        )
        nc.vector.tensor_copy(out=e8[:], in_=st8[:])

    # ---- persistent per-core buffers ----
    qfT = singles.tile([128, 4, s_total], DT_MM, name="qfT", tag="qfT")  # [inner, s]
    kvsb = singles.tile([128, 4, 128], DT_MM, name="kvsb", tag="kvsb")  # block-diag per pair
    # v' staging buffers (manual 3-deep rotation so the ones column is written once)
    vqs = [singles.tile([128, 4, 129], DT_MM, name=f"vq{i}", tag=f"vq{i}") for i in range(3)]
    for i in range(3):
        nc.vector.tensor_copy(out=vqs[i][:, :, 128:129], in_=ones_vcol[:])
    # block-diag Ksum rhs: [128, pair, 8]; pair c: rows 0-63 -> col 2c, rows 64-127 -> col 2c+1
    ksum_bd = singles.tile([128, 4, 8], DT_MM, name="ksum_bd", tag="ksum_bd")
    if True:
        kbz = ctx.enter_context(tc.tile_pool(name="kbz_stage", bufs=2))
        stz = kbz.tile([128, 4, 8], F32, tag="stz")
        nc.gpsimd.memset(stz[:], 0.0)
        nc.vector.tensor_copy(out=ksum_bd[:], in_=stz[:])
        stz2 = kbz.tile([128, 4, 128], F32, tag="stz2")
        nc.gpsimd.memset(stz2[:], 0.0)
        nc.vector.tensor_copy(out=kvsb[:], in_=stz2[:])

    def _passes():
        # =================== PASS A ===================
        with ExitStack() as actx:
            x_pool = actx.enter_context(tc.tile_pool(name="x_pool", bufs=2))
            xbf_pool = actx.enter_context(tc.tile_pool(name="xbf_pool", bufs=3))
            xT_pool = actx.enter_context(tc.tile_pool(name="xT_pool", bufs=2))
            er_pool = actx.enter_context(tc.tile_pool(name="er_pool", bufs=6))
            kf_pool = actx.enter_context(tc.tile_pool(name="kf_pool", bufs=3))
            v_pool = actx.enter_context(tc.tile_pool(name="v_pool", bufs=3))
            ps_a = actx.enter_context(tc.tile_pool(name="ps_a", bufs=4, space="PSUM"))
            ps_acc = actx.enter_context(tc.tile_pool(name="ps_acc", bufs=1, space="PSUM"))

            # KV accumulators per head pair (one PSUM bank each — interleaved
            # accumulation groups must not share a bank):
            # kvq[j] = cols 0-127 pair j's v cols, col 128 = Ksum
            kvq = [
                ps_acc.tile([128, 129], F32, name=f"kvq_{j}", tag=f"kvq_{j}")[:]
                for j in range(4)
            ]

            xT_live = {}

            def transpose_stage(ich):
                # ---- one 1MB DMA for the whole 512-row chunk ----
                xc = x_pool.tile([128, 4, D], F32, tag="x")
                nc.sync.dma_start(
                    out=xc[:],
                    in_=x_d[ich * 512 : (ich + 1) * 512, :].rearrange("(t p) d -> p t d", p=128),
                )
                xT_t = xT_pool.tile([128, 4, 512], DT_MM, tag="xT")
                # ---- cast to bf16 on ACT, transpose chunk on PE (1 cyc/row) ----
                for it in range(4):
                    xbf = xbf_pool.tile([128, D], DT_MM, tag="xbf")
                    nc.scalar.activation(xbf[:], xc[:, it, :], AF.Copy)
                    xps = ps_a.tile([128, 4, 128], DT_MM, tag="ps")
                    for c in range(4):
                        nc.tensor.transpose(xps[:, c, :], xbf[:, c * 128 : (c + 1) * 128], ident[:])
                    nc.vector.tensor_copy(out=xT_t[:, :, it * 128 : (it + 1) * 128], in_=xps[:])
                xT_live[ich] = xT_t

            def qkv_stage(ich):
                xT_t = xT_live.pop(ich)
                # ---- qT = Wq^T xT, phi -> QfT ----
                for ci in range(4):
                    qps = ps_a.tile([128, 512], F32, tag="ps")
                    for cd in range(4):
                        nc.tensor.matmul(
                            qps[:],
                            lhsT=w_sb["Wq"][:, cd, ci * 128 : (ci + 1) * 128],
                            rhs=xT_t[:, cd, :],
                            start=(cd == 0),
                            stop=(cd == 3),
                        )
                    e_t = er_pool.tile([128, 512], F32, tag="er")
                    r_t = er_pool.tile([128, 512], F32, tag="er")
                    nc.scalar.activation(e_t[:], qps[:], AF.Exp, bias=bq_sb[:, ci : ci + 1], scale=1.0)
                    nc.scalar.activation(r_t[:], qps[:], AF.Relu, bias=bq_sb[:, ci : ci + 1], scale=1.0)
                    # phi = min(exp(x),1) + relu(x)
                    nc.vector.scalar_tensor_tensor(
                        out=qfT[:, ci, ich * 512 : (ich + 1) * 512],
                        in0=e_t[:],
                        scalar=1.0,
                        in1=r_t[:],
                        op0=ALU.min,
                        op1=ALU.add,
                    )
                # ---- k, v, KV accumulation per s-tile, one-tile K lookahead:
                # K(it+1)'s matmuls fill the PE while kf(it) transits the
                # ACT exp/relu -> DVE stt chain, so the KV matmuls never stall
                # (stalls also drop the PE out of its boosted p-state).
                er_live, kf_live = {}, {}

                def emit_K_mm(it):
                    # k (natural layout) + bias via ones-row matmul
                    kps = ps_a.tile([128, 512], F32, tag="ps")
                    for cd in range(4):
                        nc.tensor.matmul(
                            kps[:],
                            lhsT=xT_t[:, cd, it * 128 : (it + 1) * 128],
                            rhs=w_sb["Wk"][:, cd, :],
                            start=(cd == 0),
                            stop=False,
                        )
                    nc.tensor.matmul(
                        kps[:], lhsT=ones_col[:], rhs=bk_row, start=False, stop=True
                    )
                    e_t = er_pool.tile([128, 512], F32, tag="er")
                    r_t = er_pool.tile([128, 512], F32, tag="er")
                    nc.scalar.activation(e_t[:], kps[:], AF.Exp)
                    nc.scalar.activation(r_t[:], kps[:], AF.Relu)
                    er_live[it] = (e_t, r_t)

                def emit_K_stt(it):
                    e_t, r_t = er_live.pop(it)
                    kf = kf_pool.tile([128, 512], DT_MM, tag="kf")
                    nc.vector.scalar_tensor_tensor(
                        out=kf[:], in0=e_t[:], scalar=1.0, in1=r_t[:], op0=ALU.min, op1=ALU.add
                    )
                    kf_live[it] = kf

                def emit_VKV(it):
                    ist = ich * 4 + it
                    first, last = (ist == 0), (ist == NT - 1)
                    # v (natural) with bias fused into the psum->sbuf copy
                    vps = ps_a.tile([128, 512], F32, tag="ps")
                    for cd in range(4):
                        nc.tensor.matmul(
                            vps[:],
                            lhsT=xT_t[:, cd, it * 128 : (it + 1) * 128],
                            rhs=w_sb["Wv"][:, cd, :],
                            start=(cd == 0),
                            stop=(cd == 3),
                        )
                    vq = vqs[ist % 3]
                    nc.vector.tensor_add(
                        out=vq[:, :, 0:128],
                        in0=vps[:].rearrange("p (g n) -> p g n", g=4),
                        in1=bv_rep[:].rearrange("p (g n) -> p g n", g=4),
                    )
                    kf = kf_live.pop(it)
                    # KV accumulation per head pair
                    for j in range(4):
                        nc.tensor.matmul(
                            kvq[j],
                            lhsT=kf[:, j * 128 : (j + 1) * 128],
                            rhs=vq[:, j, :],
                            start=first,
                            stop=last,
                        )

                emit_K_mm(0)
                emit_K_stt(0)
                for it in range(4):
                    if it + 1 < 4:
                        emit_K_mm(it + 1)
                    emit_VKV(it)
                    if it + 1 < 4:
                        emit_K_stt(it + 1)

            # software pipeline: transposes run one chunk ahead of Q/K/V so the
            # PE has work while weights load and ACT casts the next chunk
            for ich in range(NCH + 1):
                if ich < NCH:
                    transpose_stage(ich)
                if ich == 0:
                    # late-emitted weights: their DVE casts queue behind the
                    # chunk-0 xT copies instead of ahead of them
                    emit_weight("Wv")
                    emit_weight("Wo")
                if ich >= 1:
                    qkv_stage(ich - 1)

            # ---- extract KV blocks and Ksum (still inside pass-A pool scope) ----
            for h in range(H):
                j, rh = h // 2, (h % 2) * 64
                nc.vector.tensor_copy(
                    out=kvsb[rh : rh + 64, j, rh : rh + 64],
                    in_=kvq[j][rh : rh + 64, rh : rh + 64],
                )
            for c in range(4):
                for half in range(2):
                    nc.vector.tensor_copy(
                        out=ksum_bd[half * 64 : (half + 1) * 64, c, 2 * c + half : 2 * c + half + 1],
                        in_=kvq[c][half * 64 : (half + 1) * 64, 128:129],
                    )

        # ======================= PASS B =======================
        with ExitStack() as bctx:
            dz_ps = bctx.enter_context(tc.tile_pool(name="dz_ps", bufs=2, space="PSUM"))
            zrep_ps = bctx.enter_context(tc.tile_pool(name="zrep_ps", bufs=1, space="PSUM"))
            ot_ps = bctx.enter_context(tc.tile_pool(name="ot_ps", bufs=1, space="PSUM"))
            ow_ps = bctx.enter_context(tc.tile_pool(name="ow_ps", bufs=2, space="PSUM"))
            ztsb_pool = bctx.enter_context(tc.tile_pool(name="ztsb_pool", bufs=2))
            otsb_pool = bctx.enter_context(tc.tile_pool(name="otsb_pool", bufs=2))
            out_pool = bctx.enter_context(tc.tile_pool(name="out_pool", bufs=3))

            for isl in range(NSL):
                # denT [8, 512] = sum_c ksum_bd[:,c,:]^T @ qfT[:,c,slice] + eps
                dz = dz_ps.tile([8, 512], F32, tag="dz")
                for c in range(4):
                    nc.tensor.matmul(
                        dz[:],
                        lhsT=ksum_bd[:, c, :],
                        rhs=qfT[:, c, isl * 512 : (isl + 1) * 512],
                        start=(c == 0),
                        stop=False,
                    )
                nc.tensor.matmul(dz[:], lhsT=eps_col[:], rhs=ones512[:], start=False, stop=True)
                # Z = 1/den as exp(-ln(den)) on ACT (Reciprocal is banned there,
                # and DVE reciprocal costs ~6.5ns/elem — too slow at 512 free)
                lntmp = ztsb_pool.tile([8, 512], F32, tag="lntmp")
                nc.scalar.activation(lntmp[:], dz[:], AF.Ln)
                ztsb = ztsb_pool.tile([8, 512], DT_MM, tag="ztsb")
                nc.scalar.activation(ztsb[:], lntmp[:], AF.Exp, scale=-1.0)

                for half in range(2):
                    # O^T per head pair (block-diagonal KV) — emitted before zrep
                    # so the PE streams OT while ACT finishes the Z chain
                    otps = ot_ps.tile([128, 4, 256], F32, tag="ot")
                    for c in range(4):
                        nc.tensor.matmul(
                            otps[:, c, :],
                            lhsT=kvsb[:, c, :],
                            rhs=qfT[:, c, isl * 512 + half * 256 : isl * 512 + (half + 1) * 256],
                            start=True,
                            stop=True,
                        )
                    # Zrep: replicate Z rows across head d-partitions (E8 selector matmul)
                    zrep = zrep_ps.tile([128, 4, 256], F32, tag="zrep")
                    for c in range(4):
                        nc.tensor.matmul(
                            zrep[:, c, :],
                            lhsT=e8[:, c, :],
                            rhs=ztsb[:, half * 256 : (half + 1) * 256],
                            start=True,
                            stop=True,
                        )
                    # DVE may read only one PSUM operand per op: stage zrep via ACT
                    zrep_sb = otsb_pool.tile([128, 4, 256], DT_MM, tag="zrep_sb")
                    nc.scalar.activation(zrep_sb[:], zrep[:], AF.Copy)
                    otsb = otsb_pool.tile([128, 4, 256], DT_MM, tag="otsb")
                    nc.vector.tensor_mul(out=otsb[:], in0=otps[:], in1=zrep_sb[:])
                    # out = OT^T Wo; +bo fused into the psum->sbuf copy
                    for h2 in range(2):
                        ist = isl * 4 + half * 2 + h2
                        owps = ow_ps.tile([128, 512], F32, tag="ow")
                        for c in range(4):
                            nc.tensor.matmul(
                                owps[:],
                                lhsT=otsb[:, c, h2 * 128 : (h2 + 1) * 128],
                                rhs=w_sb["Wo"][:, c, :],
                                start=(c == 0),
                                stop=(c == 3),
                            )
                        outt = out_pool.tile([128, 512], F32, tag="outt")
                        nc.vector.tensor_add(out=outt[:], in0=owps[:], in1=bo_rep[:])
                        nc.sync.dma_start(out=out_d[ist * 128 : (ist + 1) * 128, :], in_=outt[:])

    if reps == 1:
        _passes()
    else:
        with tc.For_i(0, reps, 1):
            _passes()


def _legalize_waits(nc: "bass.Bass", max_waits: int = 1) -> int:
    """This toolchain's walrus allows at most ONE sync wait per instruction.

    Tile's scheduler attaches several; hoist the extras into standalone
    event-semaphore (pure wait) instructions on the same engine, placed
    immediately before the original — identical blocking semantics since
    waits execute in stream order on the issuing sequencer.
    """
    n_split = 0
    for func in nc.m.functions:
        for block in func.blocks:
            new_insts = []
            for inst in block.instructions:
                si = getattr(inst, "sync_info", None)
                waits = list(si.on_wait) if (si and si.on_wait) else []
                if len(waits) > max_waits:
                    extra, keep = waits[:-max_waits], waits[-max_waits:]
                    for j, w in enumerate(extra):
                        ev = mybir.InstEventSemaphore(
                            name=f"{inst.name}_lw{j}",
                            engine=inst.engine,
                            ins=[],
                            outs=[],
                            sync_info=mybir.SyncInfo(on_wait=[w], on_update=[]),
                        )
                        new_insts.append(ev)
                        n_split += 1
                    si.on_wait = keep
                new_insts.append(inst)
            block.instructions[:] = new_insts
    return n_split




def build_program(s_total: int = S, reps: int = 1) -> "bass.Bass":
    nc = bass.Bass("TRN2", target_bir_lowering=False, debug=False, num_devices=B)
    io = {
        "x": nc.dram_tensor("x", [s_total, D], F32, kind="ExternalInput").ap(),
        "Wq": nc.dram_tensor("Wq", [D, INNER], F32, kind="ExternalInput").ap(),
        "bq": nc.dram_tensor("bq", [INNER], F32, kind="ExternalInput").ap(),
        "Wk": nc.dram_tensor("Wk", [D, INNER], F32, kind="ExternalInput").ap(),
        "bk": nc.dram_tensor("bk", [INNER], F32, kind="ExternalInput").ap(),
        "Wv": nc.dram_tensor("Wv", [D, INNER], F32, kind="ExternalInput").ap(),
        "bv": nc.dram_tensor("bv", [INNER], F32, kind="ExternalInput").ap(),
        "Wo": nc.dram_tensor("Wo", [INNER, D], F32, kind="ExternalInput").ap(),
        "bo": nc.dram_tensor("bo", [D], F32, kind="ExternalInput").ap(),
        "out": nc.dram_tensor("out", [s_total, D], F32, kind="ExternalOutput").ap(),
    }
    with tile.TileContext(nc) as tc:
        with ExitStack() as ctx:
            _linattn_body(ctx, tc, io, s_total, reps=reps)
    return nc


_PROGRAM_CACHE: dict = {}


def _get_program(s_total: int = S) -> "bass.Bass":
    if s_total not in _PROGRAM_CACHE:
        nc = build_program(s_total)
        _legalize_waits(nc)
        _PROGRAM_CACHE[s_total] = nc
    return _PROGRAM_CACHE[s_total]


def _in_maps(inputs: dict) -> list:
    maps = []
    for b in range(B):
        m = {"x": np.ascontiguousarray(inputs["x"][b], dtype=np.float32)}
        for name in ("Wq", "bq", "Wk", "bk", "Wv", "bv", "Wo", "bo"):
            m[name] = np.ascontiguousarray(inputs[name], dtype=np.float32)
        maps.append(m)
    return maps


def run_hw(inputs: dict, trace: bool = False, **kwargs):
    """Run on the 8 NeuronCores. Returns (out [B,S,D], BassKernelResults)."""
    nc = _get_program(S)
    res = run_bass_kernel_spmd(nc, _in_maps(inputs), list(range(B)), trace=trace, **kwargs)
    out = np.stack([res.results[b]["out"] for b in range(B)], axis=0)
    return out, res


def kernel(**inputs) -> np.ndarray:
    out, _ = run_hw(inputs, trace=False)
    return out


def bench_hw(inputs: dict, iters: int = 20, nc_override=None):
    """Time repeated NEFF executions with device-resident inputs.

    Returns (per_iter_ns, out[B,S,D] from the first run). Uses the same
    shard_map lowering as run_bass_via_pjrt, without donation so input
    buffers can be reused across timed calls.
    """
    import time as _time

    import jax
    from jax.sharding import Mesh, NamedSharding, PartitionSpec
    from jax.experimental.shard_map import shard_map

    from concourse import bass2jax
    from concourse.bass2jax import _bass_exec_p, install_neuronx_cc_hook

    install_neuronx_cc_hook()
    nc = nc_override if nc_override is not None else _get_program(S)
    in_maps = _in_maps(inputs)

    partition_name = nc.partition_id_tensor.name if nc.partition_id_tensor else None
    in_names, out_names, out_avals = [], [], []
    for alloc in nc.m.functions[0].allocations:
        if not isinstance(alloc, mybir.MemoryLocationSet):
            continue
        name = alloc.memorylocations[0].name
        if alloc.kind == "ExternalInput":
            if name != partition_name:
                in_names.append(name)
        elif alloc.kind == "ExternalOutput":
            out_names.append(name)
            out_avals.append(
                jax.core.ShapedArray(tuple(alloc.tensor_shape), mybir.dt.np(alloc.dtype))
            )
    n_params = len(in_names)
    all_in_names = in_names + out_names
    if partition_name is not None:
        all_in_names = all_in_names + [partition_name]

    def _body(*args):
        operands = list(args)
        if partition_name is not None:
            operands.append(bass2jax.partition_id_tensor())
        outs = _bass_exec_p.bind(
            *operands,
            out_avals=tuple(out_avals),
            in_names=tuple(all_in_names),
            out_names=tuple(out_names),
            lowering_input_output_aliases=(),
            sim_require_finite=True,
            sim_require_nnan=True,
            nc=nc,
        )
        return tuple(outs)

    devices = jax.devices()[:B]
    mesh = Mesh(np.asarray(devices), ("core",))
    n_outs = len(out_names)
    in_specs = (PartitionSpec("core"),) * (n_params + n_outs)
    out_specs = (PartitionSpec("core"),) * n_outs
    fn = jax.jit(
        shard_map(_body, mesh=mesh, in_specs=in_specs, out_specs=out_specs, check_rep=False)
    )

    sh = NamedSharding(mesh, PartitionSpec("core"))
    concat_in = [
        jax.device_put(
            np.concatenate([np.asarray(in_maps[c][nm])[None] for c in range(B)], axis=0).reshape(
                B * np.asarray(in_maps[0][nm]).shape[0], *np.asarray(in_maps[0][nm]).shape[1:]
            ),
            sh,
        )
        for nm in in_names
    ]
    concat_zeros = [
        jax.device_put(np.zeros((B * a.shape[0], *a.shape[1:]), a.dtype), sh) for a in out_avals
    ]

    out = fn(*concat_in, *concat_zeros)
    jax.block_until_ready(out)
    first = np.asarray(out[0]).reshape(B, *out_avals[0].shape)

    def timed(f, n):
        t0 = _time.perf_counter()
        for _ in range(n):
            r = f(*concat_in, *concat_zeros)
        jax.block_until_ready(r)
        return (_time.perf_counter() - t0) / n

    timed(fn, 3)
    t = min(timed(fn, max(5, iters // 2)) for _ in range(4))
    return int(t * 1e9), first


def build_copy_program(s_total: int = S) -> "bass.Bass":
    """Same I/O signature as the real program, near-zero work: out = x."""
    nc = bass.Bass("TRN2", target_bir_lowering=False, debug=False, num_devices=B)
    io = {}
    io["x"] = nc.dram_tensor("x", [s_total, D], F32, kind="ExternalInput").ap()
    for nm, shp in (("Wq", [D, INNER]), ("bq", [INNER]), ("Wk", [D, INNER]), ("bk", [INNER]),
                    ("Wv", [D, INNER]), ("bv", [INNER]), ("Wo", [INNER, D]), ("bo", [D])):
        io[nm] = nc.dram_tensor(nm, shp, F32, kind="ExternalInput").ap()
    out_d = nc.dram_tensor("out", [s_total, D], F32, kind="ExternalOutput").ap()
    from contextlib import ExitStack as _ES
    with tile.TileContext(nc) as tc:
        with _ES() as ctx:
            pool = ctx.enter_context(tc.tile_pool(name="cp", bufs=4))
            for i in range(s_total // 128):
                t = pool.tile([128, D], F32, tag="cp")
                sl = slice(i * 128, (i + 1) * 128)
                nc.sync.dma_start(out=t[:], in_=io["x"][sl])
                nc.sync.dma_start(out=out_d[sl], in_=t[:])
    _legalize_waits(nc)
    return nc
